# revision 50
# baseline (speedup 1.0000x reference)
"""Trainium2 Bass kernel for the CustomAutoencoder problem.

Network (per batch row):
    h  = relu(einsum('gk,k->g', gw*mask, x[idx]) + gb)   # grouped gather-dense
    h  = relu(h @ w1 + b1); z = relu(h @ w2 + b2)
    d  = relu(z @ dw1 + db1); d = relu(d @ dw2 + db2)
    out = sigmoid(d @ dw3 + db3)

The gather+grouped-dense encoder is mathematically x @ A with
A[s, g] = sum_k (gw*mask)[g, k] * (idx[g, k] == s), so the whole model is a
dense MLP chain.  A is built on the host from the small param tensors
(replicated per the data-parallel sharding) and the batch dim is sharded
across 8 NeuronCores.

Datapath: x is transposed AND cast to fp8e4m3 on the host, so the device
runs a pure matmul chain with no on-device transposes.  The three big
matmuls (L1 x@A, L2 h1@w1, L6 d2@dw3) run as fp8 DoubleRow (2 contraction
rows per partition, 0.5 PE cycles/output-row); mid layers stay bf16 and all
PSUM accumulation is fp32 (measured end-to-end rel_fro ~2e-3, gate 2e-2).
db3 enters the L6 accumulation group as a ones-outer-product DoubleRow
matmul with a two-term fp8 compensation (hi + residual).  The output is
stored bf16 and upconverted on the host.

PSUM is only readable by DVE and ACT on TRN2 (GPSIMD/Pool cannot access
PSUM - BIR verifier), so the PSUM drains are balanced across those two:
DVE takes h1 pair0 / h2 / z / d2, ACT takes h1 pair1 / d1 / the sigmoids.
Each drained tile is written by exactly one engine so slice writes never
mix engines.

DoubleRow operand layout (verified in CoreSim): lhsT [p, 2, M], rhs
[p, 2, N], out[M, N] = sum_i lhsT[:, i, :].T @ rhs[:, i, :], i.e. logical
contraction k = i*P + p.

DMA: 4 loads + 4 half-block stores fit in the 8 HWDGE queues so no
queue-order wait is needed; each store is emitted right after its gating
sigmoid pair so the transfers overlap the remaining compute.  The tiny
"touch" matmuls / copies pre-advance each engine's observed vector clock
past DMA producers: walrus allows only ONE sync wait on matmul (S3_LW),
activation (S3D3_AC) and DMA instructions, so every real instruction must
need at most one semaphore wait.
"""

import os
import sys

sys.path.insert(0, "/opt/trn_rl_repo")

import numpy as np

import concourse.bass as bass
import concourse.tile as tile
from concourse import mybir
from concourse.bass import ts
from concourse.bass_utils import run_bass_kernel_spmd

F32 = mybir.dt.float32
BF = mybir.dt.bfloat16
F8 = mybir.dt.float8e4
U8 = mybir.dt.uint8
AF = mybir.ActivationFunctionType
DR = mybir.MatmulPerfMode.DoubleRow

B = 8192          # full batch
S = 512           # sample size (input/output features)
G = 510           # number of groups
GP = 512          # G padded to a multiple of 128
HID = 128
LAT = 32
HID2 = 256
NCORES = 8
BC = B // NCORES  # rows per core
BBLK = 512        # batch columns per block (PSUM free-dim max for fp32)
NBLK = BC // BBLK

# apack (fp8 bytes) [128, APR_COLS]: A+w1 in DoubleRow layout, then the
# fp32 biases and bf16 mid-layer weights embedded as raw bytes (the device
# reads them through bitcast slices).  One DMA covers everything the front
# half of the network needs.
A_BASE = 0            # 2560 cols: [P(2), i(2), gj(5), m(128)], gj=4 is w1
BIAS_BASE = 2560      # 9 fp32 bias cols (4 bytes each): +0..3 gb chunks,
                      # +4 b1, +5 b2 (rows<32), +6 db1, +7..8 db2 chunks
W2_BASE = 2596        # 128 x 32 bf16 (64 cols)
DW1_BASE = 2660       # 32 x 128 bf16 (256 cols, rows 0-31)
DW2_BASE = 2916       # 128 x 256 bf16 (512 cols)
APR_COLS = 3428

# w8pack (fp8) [128, 2, W8_FREE]: dim1 is the DoubleRow k-tile.
DW3_OFF = 0           # [:, i, 0:512]   dw3[i*128+p, s]
ONES_OFF = 512        # [0, i, 512:640] all-ones rows
DB3_OFF = 640         # [0, 0, :] = fp8(db3), [0, 1, :] = fp8(db3 - fp8(db3))
W8_FREE = 1152

_CACHE: dict = {}
last_results = None


def _touch(nc, scratch, tl):
    """1x1 PE matmul reading a corner of `tl`: advances the PE engine's
    observed vector clock past tl's producer (walrus S3_LW single-wait)."""
    return nc.tensor.matmul(
        scratch[0:1, 0:2], tl[0:1, 0:1], tl[0:1, 0:2], start=True, stop=True
    )


_DUMP_IDX = [0, 0, 0]


def _stouch(nc, sdump, tl):
    """Scalar-engine equivalent of _touch for ACT-consumed (bias) tiles."""
    k = _DUMP_IDX[0] % 32
    _DUMP_IDX[0] += 1
    return nc.scalar.copy(out=sdump[0:1, k : k + 1], in_=tl[0:1, 0:1])


def _vtouch(nc, vdump, tl):
    """Vector-engine equivalent of _touch for DVE-consumed tiles."""
    k = _DUMP_IDX[1] % 32
    _DUMP_IDX[1] += 1
    return nc.vector.tensor_copy(vdump[0:1, k : k + 1], tl[0:1, 0:1])


class SplitDrainTileContext(tile.TileContext):
    """TileContext whose kernel-tail drain carries at most one sync wait per
    instruction: this walrus build rejects >1 sync wait on any instruction,
    and the stock tail drain aggregates one wait per active proc."""

    def _drain_and_barrier(self, tick_clock, wait_clock):
        from concourse.vector_clock import ScopedClock, VectorClock

        gc = tick_clock.global_clock
        n = len(gc)
        for p in range(n):
            t = gc[p]
            if t == 0:
                continue
            single = [0] * n
            single[p] = t
            nop = self.nc.sync.nop(nofuse=True, hint="split_drain_wait")
            wait_clock.add_sem_waits(
                nop.ins, ScopedClock({None: VectorClock(single)})
            )
        # The per-proc nops above already enforce every outstanding tick in
        # SP program order, so the drain itself needs no waits.
        self.nc.sync.drain()
        self.nc.all_engine_barrier()
        assert self.sems is not None
        popped = self.nc._tile_sem_poison_stack.pop()
        assert popped is self._sem_poison
        self.nc.clear_and_free_semaphores(list(self.sems.allocated().values()))
        self.nc.all_engine_barrier()


def _build_program():
    nc = bass.Bass()

    # x arrives host-transposed [S, BC] fp8: row s = P*256 + i*128 + p.
    xt_d = nc.declare_dram_parameter("xt", [S, BC], F8, isOutput=False)
    a_d = nc.declare_dram_parameter("apack", [128, APR_COLS], U8,
                                    isOutput=False)
    w8_d = nc.declare_dram_parameter("w8pack", [128, 2, W8_FREE], F8,
                                     isOutput=False)
    out_d = nc.declare_dram_parameter("out", [BC, S], BF, isOutput=True)

    xt_v = xt_d.rearrange("(P i p) (k b) -> k p P i b", P=2, i=2, p=128,
                          b=BBLK)                       # [NBLK,128,2,2,512]
    out_v = out_d.rearrange("(k i p) s -> k p i s", p=128, i=4)

    with SplitDrainTileContext(nc) as tc:
        with (
            tc.tile_pool(name="weights", bufs=1) as wp,
            tc.tile_pool(name="xin", bufs=2) as xp,
            tc.tile_pool(name="h1", bufs=4) as h1p,
            tc.tile_pool(name="acts", bufs=4) as ap_,
            tc.tile_pool(name="d2", bufs=2) as d2p,
            tc.tile_pool(name="outs", bufs=2) as op_,
            tc.tile_pool(name="p1", bufs=2, space="PSUM") as p1p,
            tc.tile_pool(name="pmid", bufs=2, space="PSUM") as pmp,
            tc.tile_pool(name="p6", bufs=2, space="PSUM") as p6p,
            tc.tile_pool(name="p6b", bufs=1, space="PSUM") as p6bp,
            tc.tile_pool(name="psc", bufs=1, space="PSUM") as pscp,
        ):
            scratch = pscp.tile([1, 2], F32)
            sdump = wp.tile([1, 32], F32, tag="sdump")
            vdump = wp.tile([1, 32], F32, tag="vdump")
            wdump = wp.tile([1, 2], F32, tag="wdump")
            _DUMP_IDX[0] = _DUMP_IDX[1] = _DUMP_IDX[2] = 0

            # Loads in need order; SP serializes transfers in issue order.
            xts = []
            xt = xp.tile([128, 2, 2, BBLK], F8, tag="xt")
            nc.sync.dma_start(out=xt[:], in_=xt_v[0])
            xts.append(xt)
            a_sb = wp.tile([128, APR_COLS], U8, tag="apack")
            nc.sync.dma_start(out=a_sb[:], in_=a_d[:, :])
            xt = xp.tile([128, 2, 2, BBLK], F8, tag="xt")
            nc.sync.dma_start(out=xt[:], in_=xt_v[1])
            xts.append(xt)
            w8_sb = wp.tile([128, 2, W8_FREE], F8, tag="w8pack")
            nc.sync.dma_start(out=w8_sb[:], in_=w8_d[:])

            a5 = a_sb[:, A_BASE : A_BASE + 2560].bitcast(F8).rearrange(
                "p (P i g m) -> p P i g m", P=2, i=2, g=5, m=128
            )

            def a_sl(P, gj):         # lhsT [128, 2, 128], DR pair P
                return a5[:, P, :, gj, :]

            def w1_sl(P):            # lhsT [128, 2, 128]
                return a5[:, P, :, 4, :]

            w2_sl = a_sb[:, W2_BASE : W2_BASE + 2 * LAT].bitcast(BF)
            dw1_sl = a_sb[0:LAT, DW1_BASE : DW1_BASE + 2 * HID].bitcast(BF)

            def dw2_sl(j):           # lhsT [128, 128]
                c = DW2_BASE + j * 256
                return a_sb[:, c : c + 256].bitcast(BF)

            dw3_sl = w8_sb[:, :, DW3_OFF : DW3_OFF + S]       # [128, 2, 512]
            ones_sl = w8_sb[0:1, :, ONES_OFF : ONES_OFF + 128]  # [1, 2, 128]
            db3_sl = w8_sb[0:1, :, DB3_OFF : DB3_OFF + S]     # [1, 2, 512]

            def bias_col(i, rows=128):
                c = BIAS_BASE + 4 * i
                return a_sb[0:rows, c : c + 4].bitcast(F32)

            gb_b = [bias_col(i) for i in range(4)]
            b1_b = bias_col(4)
            b2_b = bias_col(5, rows=LAT)
            db1_b = bias_col(6)
            db2_b = [bias_col(7 + j) for j in range(2)]

            st_ = {"h1": {}, "h2": {}, "z": {}, "d1": {}, "d2": {}, "ob": {}}

            def relu_drain(eng, out_ap, ps, bias):
                """PSUM -> SBUF bias+relu on the chosen engine."""
                if eng == "dve":
                    nc.vector.tensor_scalar(
                        out_ap, ps[:], bias, 0.0,
                        op0=mybir.AluOpType.add, op1=mybir.AluOpType.max,
                    )
                else:
                    nc.scalar.activation(out_ap, ps[:], AF.Relu, bias=bias)

            # Drain-engine assignment: each drained tile is written by one
            # engine.  ACT carries the 8 sigmoids plus the h1 pair-1 tiles
            # and d1; DVE the rest.
            H1_ENG = {(0, 0): "dve", (0, 1): "act",
                      (1, 0): "dve", (1, 1): "act"}
            Z_ENG = {0: "dve", 1: "dve"}

            def l1(blk, gj):
                # h1T[gj] = relu(sum_s A[s, gj].T x[b, s] + gb[gj]), fp8 DR.
                ps = p1p.tile([128, BBLK], F32, tag="p1")
                for P in range(2):
                    nc.tensor.matmul(
                        ps[:], a_sl(P, gj), xts[blk][:, P, :, :],
                        start=(P == 0), stop=(P == 1), perf_mode=DR,
                    )
                if gj == 0:
                    h = h1p.tile([128, 2, BBLK], F8, tag="h1a")
                    st_["h1"].setdefault(blk, []).append(h)
                elif gj == 2:
                    h = h1p.tile([128, 2, BBLK], F8, tag="h1b")
                    st_["h1"][blk].append(h)
                h = st_["h1"][blk][gj // 2]
                relu_drain(H1_ENG[(blk, gj // 2)], h[:, gj % 2, :], ps,
                           gb_b[gj])

            def touch_h1(blk):
                # PE observes the producing engines' ticks of both h1 pair
                # tiles, so L2 matmuls keep at most one sync wait.
                for h in st_["h1"][blk]:
                    _touch(nc, scratch, h[:, 1, :])

            def l2(blk):
                ps = pmp.tile([128, BBLK], F32, tag="pmid")
                for P in range(2):
                    nc.tensor.matmul(
                        ps[:], w1_sl(P), st_["h1"][blk][P][:],
                        start=(P == 0), stop=(P == 1), perf_mode=DR,
                    )
                h2 = ap_.tile([HID, BBLK], BF, tag="h2")
                relu_drain("dve", h2[:], ps, b1_b)
                st_["h2"][blk] = h2

            def l3(blk):
                ps = pmp.tile([LAT, BBLK], F32, tag="pmid")
                nc.tensor.matmul(ps[:], w2_sl, st_["h2"][blk][:], start=True,
                                 stop=True)
                z = ap_.tile([LAT, BBLK], BF, tag="z")
                relu_drain(Z_ENG[blk], z[:], ps, b2_b)
                st_["z"][blk] = z

            def l4(blk):
                ps = pmp.tile([HID, BBLK], F32, tag="pmid")
                nc.tensor.matmul(ps[:], dw1_sl, st_["z"][blk][:], start=True,
                                 stop=True)
                d1 = ap_.tile([HID, BBLK], BF, tag="d1")
                relu_drain("act", d1[:], ps, db1_b)
                st_["d1"][blk] = d1
                # PE observes the ACT tick of d1 so L5 matmuls keep at most
                # one sync wait (their psum-slot wait).
                _touch(nc, scratch, d1)

            def l5(blk, j):
                ps = pmp.tile([128, BBLK], F32, tag="pmid")
                nc.tensor.matmul(ps[:], dw2_sl(j), st_["d1"][blk][:],
                                 start=True, stop=True)
                if j == 0:
                    d2 = d2p.tile([128, 2, BBLK], F8, tag="d2")
                    st_["d2"][blk] = d2
                d2 = st_["d2"][blk]
                relu_drain("dve", d2[:, j, :], ps, db2_b[j])

            def l6(blk, bi, pool=None):
                # out[bi] = sigmoid(d2T[:, bi].T @ dw3 + db3), natural
                # layout; both the data matmul and the db3 broadcast
                # (ones-outer-product, hi+residual fp8 compensation) are
                # fp8 DoubleRow in one accumulation group.  Late l6 groups
                # borrow the (by then idle) pmid banks so four sigmoid
                # psums are in flight at once.
                if bi == 0:
                    ob = op_.tile([128, 4, S], BF, tag="ob")
                    st_["ob"][blk] = ob
                ps = (pool or p6p).tile([128, S], F32, tag="p6")
                nc.tensor.matmul(
                    ps[:], st_["d2"][blk][:, :, ts(bi, 128)], dw3_sl,
                    start=True, stop=False, perf_mode=DR,
                )
                nc.tensor.matmul(ps[:], ones_sl, db3_sl, start=False,
                                 stop=True, perf_mode=DR)
                nc.scalar.activation(st_["ob"][blk][:, bi, :], ps[:],
                                     AF.Sigmoid)

            def store(blk, h):
                # Half-block HWDGE stores, emitted right after the gating
                # sigmoid pair so the transfers overlap remaining compute.
                ob = st_["ob"][blk]
                nc.sync.dma_start(out=out_v[blk, :, 2 * h : 2 * h + 2],
                                  in_=ob[:, 2 * h : 2 * h + 2])

            # Software-pipelined emission: per-engine streams execute in
            # program order, so blk1's L1 is interleaved into blk0's
            # mid-layer chain, and the two blocks' L6/sigmoid groups are
            # interleaved so the final store's gating sigmoid runs early.
            # The first ACT op has no data deps: it absorbs the one-time
            # activation-table load (~1.3us) before real work arrives.
            # (DVE memset first so the source is initialized for CoreSim.)
            nc.vector.memset(wdump[0:1, 0:2], 0.0)
            _stouch(nc, sdump, wdump[0:1, 0:1])
            _touch(nc, scratch, xts[0][:, 0, 0, :])
            a8v = a_sb[:, 0:2560].bitcast(F8)
            _touch(nc, scratch, a8v)
            _vtouch(nc, vdump, a8v[:, 0:1])
            _stouch(nc, sdump, a8v[:, 0:1])
            for gj in range(4):
                l1(0, gj)
            _touch(nc, scratch, xts[1][:, 0, 0, :])
            for gj in range(4):
                l1(1, gj)
            touch_h1(0)
            l2(0)
            l3(0)
            l4(0)
            l5(0, 0)
            l5(0, 1)
            touch_h1(1)
            l2(1)
            l3(1)
            l4(1)
            l5(1, 0)
            l5(1, 1)
            _touch(nc, scratch, w8_sb[:, 0, :])
            _touch(nc, scratch, st_["d2"][0][:, 1, :])
            l6(0, 0)
            l6(0, 1)
            store(0, 0)
            _touch(nc, scratch, st_["d2"][1][:, 1, :])
            l6(1, 0, pool=p6bp)
            l6(1, 1)
            store(1, 0)
            l6(0, 2)
            l6(0, 3, pool=p6bp)
            store(0, 1)
            l6(1, 2)
            l6(1, 3)
            store(1, 1)

    return nc


def _get_program():
    if "prog" not in _CACHE:
        _CACHE["prog"] = _build_program()
    return _CACHE["prog"]


def _to_bf16(a):
    """Round-to-nearest-even fp32 -> bf16, returned as a uint16 view."""
    u = np.ascontiguousarray(a, dtype=np.float32).view(np.uint32)
    return (((u >> 16) + ((u >> 15) & 1)) & 0xFFFF).astype(np.uint16)


def _bf16_arr(u16):
    import ml_dtypes

    return u16.view(ml_dtypes.bfloat16)


def _f8(a):
    import ml_dtypes

    return np.asarray(a, dtype=np.float32).astype(ml_dtypes.float8_e4m3)


def _put_f32(pack_u16, rows, col2, vals):
    """Embed fp32 values as adjacent uint16 pairs at bf16 column col2."""
    v = np.asarray(vals, dtype=np.float32).view(np.uint32)
    pack_u16[:rows, col2] = (v & 0xFFFF).astype(np.uint16)
    pack_u16[:rows, col2 + 1] = (v >> 16).astype(np.uint16)


def _pack_params(inputs):
    import ml_dtypes

    gw = np.asarray(inputs["gw"], dtype=np.float32)
    gb = np.asarray(inputs["gb"], dtype=np.float32)
    idx = np.asarray(inputs["idx"], dtype=np.int64)
    mask = np.asarray(inputs["mask"], dtype=np.float32)
    w1 = np.asarray(inputs["w1"], dtype=np.float32)
    b1 = np.asarray(inputs["b1"], dtype=np.float32)
    w2 = np.asarray(inputs["w2"], dtype=np.float32)
    b2 = np.asarray(inputs["b2"], dtype=np.float32)
    dw1 = np.asarray(inputs["dw1"], dtype=np.float32)
    db1 = np.asarray(inputs["db1"], dtype=np.float32)
    dw2 = np.asarray(inputs["dw2"], dtype=np.float32)
    db2 = np.asarray(inputs["db2"], dtype=np.float32)
    dw3 = np.asarray(inputs["dw3"], dtype=np.float32)
    db3 = np.asarray(inputs["db3"], dtype=np.float32)

    g, k = idx.shape
    assert g == G

    # Fold gather + grouped Dense(1) into a dense [S, GP] matrix.
    a_mat = np.zeros((S, GP), dtype=np.float32)
    gwm = (gw * mask).astype(np.float32)
    cols = np.repeat(np.arange(g, dtype=np.int64), k)
    np.add.at(a_mat, (idx.reshape(-1), cols), gwm.reshape(-1))

    # apack raw bytes [p, APR_COLS]: A+w1 fp8 in DoubleRow layout ([P, i,
    # gj|4=w1, m] with rows s/g = P*256 + i*128 + p), then fp32 biases and
    # bf16 mid weights as raw bytes.
    apack = np.zeros((128, APR_COLS), dtype=np.uint8)
    a8 = _f8(a_mat).reshape(2, 2, 128, 4, 128)        # [P, i, p, gj, m]
    w1_pad = np.zeros((GP, HID), dtype=np.float32)
    w1_pad[:g] = w1
    w18 = _f8(w1_pad).reshape(2, 2, 128, 128)         # [P, i, p, m]
    a_full = np.zeros((128, 2, 2, 5, 128), dtype=ml_dtypes.float8_e4m3)
    a_full[:, :, :, 0:4, :] = a8.transpose(2, 0, 1, 3, 4)
    a_full[:, :, :, 4, :] = w18.transpose(2, 0, 1, 3)
    apack[:, A_BASE : A_BASE + 2560] = \
        a_full.reshape(128, 2560).view(np.uint8)

    def put_f32_bytes(col, vals, rows=128):
        v = np.asarray(vals, dtype=np.float32)
        apack[:rows, col : col + 4] = v.view(np.uint8).reshape(rows, 4)

    gb_pad = np.zeros(GP, np.float32)
    gb_pad[:g] = gb
    for i in range(4):
        put_f32_bytes(BIAS_BASE + 4 * i, gb_pad[i * 128 : (i + 1) * 128])
    put_f32_bytes(BIAS_BASE + 16, b1)
    put_f32_bytes(BIAS_BASE + 20, b2, rows=LAT)
    put_f32_bytes(BIAS_BASE + 24, db1)
    put_f32_bytes(BIAS_BASE + 28, db2[:128])
    put_f32_bytes(BIAS_BASE + 32, db2[128:])
    apack[:, W2_BASE : W2_BASE + 2 * LAT] = \
        _to_bf16(w2).view(np.uint8).reshape(128, 2 * LAT)
    apack[:LAT, DW1_BASE : DW1_BASE + 2 * HID] = \
        _to_bf16(dw1).view(np.uint8).reshape(LAT, 2 * HID)
    apack[:, DW2_BASE : DW2_BASE + 512] = \
        _to_bf16(dw2).view(np.uint8).reshape(128, 512)

    # w8pack [p, i, free]: dw3 rows k = i*128 + p; ones + compensated db3
    # rows on partition 0.
    w8 = np.zeros((128, 2, W8_FREE), dtype=ml_dtypes.float8_e4m3)
    dw38 = _f8(dw3).reshape(2, 128, S)                # [i, p, s]
    w8[:, :, DW3_OFF : DW3_OFF + S] = dw38.transpose(1, 0, 2)
    w8[0, :, ONES_OFF : ONES_OFF + 128] = np.float32(1.0)
    db3_hi = _f8(db3)
    db3_lo = _f8(db3 - db3_hi.astype(np.float32))
    w8[0, 0, DB3_OFF : DB3_OFF + S] = db3_hi
    w8[0, 1, DB3_OFF : DB3_OFF + S] = db3_lo

    return apack, w8


def kernel(**inputs) -> np.ndarray:
    global last_results

    x = np.asarray(inputs["x"], dtype=np.float32)
    assert x.shape == (B, S), x.shape
    x8 = _f8(x)
    apack, w8pack = _pack_params(inputs)

    in_maps = [
        {"xt": np.ascontiguousarray(x8[c * BC : (c + 1) * BC].T),
         "apack": apack, "w8pack": w8pack}
        for c in range(NCORES)
    ]

    nc = _get_program()
    trace = os.environ.get("KERNEL_TRACE", "0") == "1"
    res = run_bass_kernel_spmd(nc, in_maps, list(range(NCORES)), trace=trace)
    last_results = res
    out = np.concatenate(
        [np.asarray(r["out"]) for r in res.results], axis=0
    )
    return out.astype(np.float32)


if __name__ == "__main__":
    rng = np.random.RandomState(0)
    demo = {
        "x": rng.rand(B, S).astype(np.float32),
        "gw": rng.randn(G, 30).astype(np.float32) * 0.18,
        "gb": rng.randn(G).astype(np.float32) * 0.1,
        "idx": rng.randint(0, S, (G, 30)).astype(np.int32),
        "mask": (rng.rand(G, 30) > 0.5).astype(np.float32),
        "w1": rng.randn(G, HID).astype(np.float32) * 0.04,
        "b1": rng.randn(HID).astype(np.float32) * 0.1,
        "w2": rng.randn(HID, LAT).astype(np.float32) * 0.09,
        "b2": rng.randn(LAT).astype(np.float32) * 0.1,
        "dw1": rng.randn(LAT, HID).astype(np.float32) * 0.18,
        "db1": rng.randn(HID).astype(np.float32) * 0.1,
        "dw2": rng.randn(HID, HID2).astype(np.float32) * 0.09,
        "db2": rng.randn(HID2).astype(np.float32) * 0.1,
        "dw3": rng.randn(HID2, S).astype(np.float32) * 0.06,
        "db3": rng.randn(S).astype(np.float32) * 0.1,
    }
    out = kernel(**demo)

    # numpy check of the dense-folded network
    x = demo["x"].astype(np.float64)
    a_mat = np.zeros((S, GP))
    gwm = demo["gw"] * demo["mask"]
    cols = np.repeat(np.arange(G), 30)
    np.add.at(a_mat, (demo["idx"].reshape(-1).astype(np.int64), cols),
              gwm.reshape(-1).astype(np.float64))
    gb_pad = np.zeros(GP)
    gb_pad[:G] = demo["gb"]
    w1_pad = np.zeros((GP, HID))
    w1_pad[:G] = demo["w1"]
    relu = lambda v: np.maximum(v, 0)
    h = relu(x @ a_mat + gb_pad)
    h = relu(h @ w1_pad + demo["b1"])
    z = relu(h @ demo["w2"] + demo["b2"])
    d = relu(z @ demo["dw1"] + demo["db1"])
    d = relu(d @ demo["dw2"] + demo["db2"])
    exp = 1 / (1 + np.exp(-(d @ demo["dw3"] + demo["db3"])))
    err = np.linalg.norm(out - exp) / np.linalg.norm(exp)
    print("out", out.shape, out.dtype, float(out.mean()))
    print("demo rel_fro vs numpy:", err)


# revision 54
# speedup vs baseline: 1.0321x; 1.0321x over previous
"""Trainium2 Bass kernel for the CustomAutoencoder problem.

Network (per batch row):
    h  = relu(einsum('gk,k->g', gw*mask, x[idx]) + gb)   # grouped gather-dense
    h  = relu(h @ w1 + b1); z = relu(h @ w2 + b2)
    d  = relu(z @ dw1 + db1); d = relu(d @ dw2 + db2)
    out = sigmoid(d @ dw3 + db3)

The gather+grouped-dense encoder is mathematically x @ A with
A[s, g] = sum_k (gw*mask)[g, k] * (idx[g, k] == s), so the whole model is a
dense MLP chain.  A is built on the host from the small param tensors
(replicated per the data-parallel sharding) and the batch dim is sharded
across 8 NeuronCores.

Datapath: x is transposed AND cast to fp8e4m3 on the host, so the device
runs a pure matmul chain with no on-device transposes.  The three big
matmuls (L1 x@A, L2 h1@w1, L6 d2@dw3) run as fp8 DoubleRow (2 contraction
rows per partition, 0.5 PE cycles/output-row); mid layers stay bf16 and all
PSUM accumulation is fp32 (measured end-to-end rel_fro ~2e-3, gate 2e-2).
db3 enters the L6 accumulation group as a ones-outer-product DoubleRow
matmul with a two-term fp8 compensation (hi + residual).  The output is
stored bf16 and upconverted on the host.

PSUM is only readable by DVE and ACT on TRN2 (GPSIMD/Pool cannot access
PSUM - BIR verifier), so the PSUM drains are balanced across those two:
DVE takes h1 pair0 / h2 / z / d2, ACT takes h1 pair1 / d1 / the sigmoids.
Each drained tile is written by exactly one engine so slice writes never
mix engines.

DoubleRow operand layout (verified in CoreSim): lhsT [p, 2, M], rhs
[p, 2, N], out[M, N] = sum_i lhsT[:, i, :].T @ rhs[:, i, :], i.e. logical
contraction k = i*P + p.

DMA: 4 loads + 4 half-block stores fit in the 8 HWDGE queues so no
queue-order wait is needed; each store is emitted right after its gating
sigmoid pair so the transfers overlap the remaining compute.  The tiny
"touch" matmuls / copies pre-advance each engine's observed vector clock
past DMA producers: walrus allows only ONE sync wait on matmul (S3_LW),
activation (S3D3_AC) and DMA instructions, so every real instruction must
need at most one semaphore wait.
"""

import os
import sys

sys.path.insert(0, "/opt/trn_rl_repo")

import numpy as np

import concourse.bass as bass
import concourse.tile as tile
from concourse import mybir
from concourse.bass import ts
from concourse.bass_utils import run_bass_kernel_spmd

F32 = mybir.dt.float32
BF = mybir.dt.bfloat16
F8 = mybir.dt.float8e4
U8 = mybir.dt.uint8
AF = mybir.ActivationFunctionType
DR = mybir.MatmulPerfMode.DoubleRow

B = 8192          # full batch
S = 512           # sample size (input/output features)
G = 510           # number of groups
GP = 512          # G padded to a multiple of 128
HID = 128
LAT = 32
HID2 = 256
NCORES = 8
BC = B // NCORES  # rows per core
BBLK = 512        # batch columns per block (PSUM free-dim max for fp32)
NBLK = BC // BBLK

# apack (fp8 bytes) [128, APR_COLS]: A+w1 in DoubleRow layout, then the
# fp32 biases and bf16 mid-layer weights embedded as raw bytes (the device
# reads them through bitcast slices).  One DMA covers everything the front
# half of the network needs.
A_BASE = 0            # 2560 cols: [P(2), i(2), gj(5), m(128)], gj=4 is w1
BIAS_BASE = 2560      # 9 fp32 bias cols (4 bytes each): +0..3 gb chunks,
                      # +4 b1, +5 b2 (rows<32), +6 db1, +7..8 db2 chunks
W2_BASE = 2596        # 128 x 32 bf16 (64 cols)
DW1_BASE = 2660       # 32 x 128 bf16 (256 cols, rows 0-31)
DW2_BASE = 2916       # 128 x 256 bf16 (512 cols)
APR_COLS = 3428

# w8pack (fp8) [128, 2, W8_FREE]: dim1 is the DoubleRow k-tile.
DW3_OFF = 0           # [:, i, 0:512]   dw3[i*128+p, s]
ONES_OFF = 512        # [0, i, 512:640] all-ones rows
DB3_OFF = 640         # [0, 0, :] = fp8(db3), [0, 1, :] = fp8(db3 - fp8(db3))
W8_FREE = 1152

_CACHE: dict = {}
last_results = None


def _touch(nc, scratch, tl):
    """1x1 PE matmul reading a corner of `tl`: advances the PE engine's
    observed vector clock past tl's producer (walrus S3_LW single-wait)."""
    return nc.tensor.matmul(
        scratch[0:1, 0:2], tl[0:1, 0:1], tl[0:1, 0:2], start=True, stop=True
    )


_DUMP_IDX = [0, 0, 0]


def _stouch(nc, sdump, tl):
    """Scalar-engine equivalent of _touch for ACT-consumed (bias) tiles."""
    k = _DUMP_IDX[0] % 32
    _DUMP_IDX[0] += 1
    return nc.scalar.copy(out=sdump[0:1, k : k + 1], in_=tl[0:1, 0:1])


def _vtouch(nc, vdump, tl):
    """Vector-engine equivalent of _touch for DVE-consumed tiles."""
    k = _DUMP_IDX[1] % 32
    _DUMP_IDX[1] += 1
    return nc.vector.tensor_copy(vdump[0:1, k : k + 1], tl[0:1, 0:1])


class SplitDrainTileContext(tile.TileContext):
    """TileContext whose kernel-tail drain carries at most one sync wait per
    instruction: this walrus build rejects >1 sync wait on any instruction,
    and the stock tail drain aggregates one wait per active proc."""

    def _drain_and_barrier(self, tick_clock, wait_clock):
        from concourse.vector_clock import ScopedClock, VectorClock

        gc = tick_clock.global_clock
        n = len(gc)
        for p in range(n):
            t = gc[p]
            if t == 0:
                continue
            single = [0] * n
            single[p] = t
            nop = self.nc.sync.nop(nofuse=True, hint="split_drain_wait")
            wait_clock.add_sem_waits(
                nop.ins, ScopedClock({None: VectorClock(single)})
            )
        # The per-proc nops above already enforce every outstanding tick in
        # SP program order, so the drain itself needs no waits.
        self.nc.sync.drain()
        self.nc.all_engine_barrier()
        assert self.sems is not None
        popped = self.nc._tile_sem_poison_stack.pop()
        assert popped is self._sem_poison
        self.nc.clear_and_free_semaphores(list(self.sems.allocated().values()))
        self.nc.all_engine_barrier()


def _build_program():
    nc = bass.Bass()

    # x arrives host-transposed [S, BC] fp8: row s = P*256 + i*128 + p.
    xt_d = nc.declare_dram_parameter("xt", [S, BC], F8, isOutput=False)
    a_d = nc.declare_dram_parameter("apack", [128, APR_COLS], U8,
                                    isOutput=False)
    w8_d = nc.declare_dram_parameter("w8pack", [128, 2, W8_FREE], F8,
                                     isOutput=False)
    out_d = nc.declare_dram_parameter("out", [BC, S], BF, isOutput=True)

    xt_v = xt_d.rearrange("(P i p) (k b) -> k p P i b", P=2, i=2, p=128,
                          b=BBLK)                       # [NBLK,128,2,2,512]
    out_v = out_d.rearrange("(k i p) s -> k p i s", p=128, i=4)

    with SplitDrainTileContext(nc) as tc:
        with (
            tc.tile_pool(name="weights", bufs=1) as wp,
            tc.tile_pool(name="xin", bufs=2) as xp,
            tc.tile_pool(name="h1", bufs=4) as h1p,
            tc.tile_pool(name="acts", bufs=4) as ap_,
            tc.tile_pool(name="d2", bufs=2) as d2p,
            tc.tile_pool(name="outs", bufs=2) as op_,
            tc.tile_pool(name="p1", bufs=3, space="PSUM") as p1p,
            tc.tile_pool(name="pmid", bufs=2, space="PSUM") as pmp,
            tc.tile_pool(name="p6", bufs=2, space="PSUM") as p6p,
            tc.tile_pool(name="psc", bufs=1, space="PSUM") as pscp,
        ):
            scratch = pscp.tile([1, 2], F32)
            sdump = wp.tile([1, 32], F32, tag="sdump")
            vdump = wp.tile([1, 32], F32, tag="vdump")
            wdump = wp.tile([1, 2], F32, tag="wdump")
            _DUMP_IDX[0] = _DUMP_IDX[1] = _DUMP_IDX[2] = 0

            # Loads in need order; SP serializes transfers in issue order.
            xts = []
            xt = xp.tile([128, 2, 2, BBLK], F8, tag="xt")
            nc.sync.dma_start(out=xt[:], in_=xt_v[0])
            xts.append(xt)
            a_sb = wp.tile([128, APR_COLS], U8, tag="apack")
            nc.sync.dma_start(out=a_sb[:], in_=a_d[:, :])
            xt = xp.tile([128, 2, 2, BBLK], F8, tag="xt")
            nc.sync.dma_start(out=xt[:], in_=xt_v[1])
            xts.append(xt)
            w8_sb = wp.tile([128, 2, W8_FREE], F8, tag="w8pack")
            nc.sync.dma_start(out=w8_sb[:], in_=w8_d[:])

            a5 = a_sb[:, A_BASE : A_BASE + 2560].bitcast(F8).rearrange(
                "p (P i g m) -> p P i g m", P=2, i=2, g=5, m=128
            )

            def a_sl(P, gj):         # lhsT [128, 2, 128], DR pair P
                return a5[:, P, :, gj, :]

            def w1_sl(P):            # lhsT [128, 2, 128]
                return a5[:, P, :, 4, :]

            w2_sl = a_sb[:, W2_BASE : W2_BASE + 2 * LAT].bitcast(BF)
            dw1_sl = a_sb[0:LAT, DW1_BASE : DW1_BASE + 2 * HID].bitcast(BF)

            def dw2_sl(j):           # lhsT [128, 128]
                c = DW2_BASE + j * 256
                return a_sb[:, c : c + 256].bitcast(BF)

            dw3_sl = w8_sb[:, :, DW3_OFF : DW3_OFF + S]       # [128, 2, 512]
            ones_sl = w8_sb[0:1, :, ONES_OFF : ONES_OFF + 128]  # [1, 2, 128]
            db3_sl = w8_sb[0:1, :, DB3_OFF : DB3_OFF + S]     # [1, 2, 512]

            def bias_col(i, rows=128):
                c = BIAS_BASE + 4 * i
                return a_sb[0:rows, c : c + 4].bitcast(F32)

            gb_b = [bias_col(i) for i in range(4)]
            b1_b = bias_col(4)
            b2_b = bias_col(5, rows=LAT)
            db1_b = bias_col(6)
            db2_b = [bias_col(7 + j) for j in range(2)]

            st_ = {"h1": {}, "h2": {}, "z": {}, "d1": {}, "d2": {}, "ob": {}}

            def relu_drain(eng, out_ap, ps, bias):
                """PSUM -> SBUF bias+relu on the chosen engine."""
                if eng == "dve":
                    nc.vector.tensor_scalar(
                        out_ap, ps[:], bias, 0.0,
                        op0=mybir.AluOpType.add, op1=mybir.AluOpType.max,
                    )
                else:
                    nc.scalar.activation(out_ap, ps[:], AF.Relu, bias=bias)

            # Drain-engine assignment: each drained tile is written by one
            # engine.  ACT carries the 8 sigmoids plus the h1 pair-1 tiles
            # and d1; DVE the rest.
            H1_ENG = {(0, 0): "dve", (0, 1): "act",
                      (1, 0): "dve", (1, 1): "act"}
            Z_ENG = {0: "dve", 1: "dve"}

            def l1(blk, gj):
                # h1T[gj] = relu(sum_s A[s, gj].T x[b, s] + gb[gj]), fp8 DR.
                ps = p1p.tile([128, BBLK], F32, tag="p1")
                for P in range(2):
                    nc.tensor.matmul(
                        ps[:], a_sl(P, gj), xts[blk][:, P, :, :],
                        start=(P == 0), stop=(P == 1), perf_mode=DR,
                    )
                if gj == 0:
                    h = h1p.tile([128, 2, BBLK], F8, tag="h1a")
                    st_["h1"].setdefault(blk, []).append(h)
                elif gj == 2:
                    h = h1p.tile([128, 2, BBLK], F8, tag="h1b")
                    st_["h1"][blk].append(h)
                h = st_["h1"][blk][gj // 2]
                relu_drain(H1_ENG[(blk, gj // 2)], h[:, gj % 2, :], ps,
                           gb_b[gj])

            def touch_h1(blk):
                # PE observes the producing engines' ticks of both h1 pair
                # tiles, so L2 matmuls keep at most one sync wait.
                for h in st_["h1"][blk]:
                    _touch(nc, scratch, h[:, 1, :])

            def l2(blk):
                ps = pmp.tile([128, BBLK], F32, tag="pmid")
                for P in range(2):
                    nc.tensor.matmul(
                        ps[:], w1_sl(P), st_["h1"][blk][P][:],
                        start=(P == 0), stop=(P == 1), perf_mode=DR,
                    )
                h2 = ap_.tile([HID, BBLK], BF, tag="h2")
                relu_drain("dve", h2[:], ps, b1_b)
                st_["h2"][blk] = h2

            def l3(blk):
                ps = pmp.tile([LAT, BBLK], F32, tag="pmid")
                nc.tensor.matmul(ps[:], w2_sl, st_["h2"][blk][:], start=True,
                                 stop=True)
                z = ap_.tile([LAT, BBLK], BF, tag="z")
                relu_drain(Z_ENG[blk], z[:], ps, b2_b)
                st_["z"][blk] = z

            def l4(blk):
                ps = pmp.tile([HID, BBLK], F32, tag="pmid")
                nc.tensor.matmul(ps[:], dw1_sl, st_["z"][blk][:], start=True,
                                 stop=True)
                d1 = ap_.tile([HID, BBLK], BF, tag="d1")
                relu_drain("act", d1[:], ps, db1_b)
                st_["d1"][blk] = d1
                # PE observes the ACT tick of d1 so L5 matmuls keep at most
                # one sync wait (their psum-slot wait).
                _touch(nc, scratch, d1)

            def l5(blk, j):
                ps = pmp.tile([128, BBLK], F32, tag="pmid")
                nc.tensor.matmul(ps[:], dw2_sl(j), st_["d1"][blk][:],
                                 start=True, stop=True)
                if j == 0:
                    d2 = d2p.tile([128, 2, BBLK], F8, tag="d2")
                    st_["d2"][blk] = d2
                d2 = st_["d2"][blk]
                relu_drain("dve", d2[:, j, :], ps, db2_b[j])

            def l6(blk, bi, pool=None):
                # out[bi] = sigmoid(d2T[:, bi].T @ dw3 + db3), natural
                # layout; both the data matmul and the db3 broadcast
                # (ones-outer-product, hi+residual fp8 compensation) are
                # fp8 DoubleRow in one accumulation group.  Late l6 groups
                # borrow the (by then idle) pmid banks so four sigmoid
                # psums are in flight at once.
                if bi == 0:
                    ob = op_.tile([128, 4, S], BF, tag="ob")
                    st_["ob"][blk] = ob
                ps = (pool or p6p).tile([128, S], F32, tag="p6")
                nc.tensor.matmul(
                    ps[:], st_["d2"][blk][:, :, ts(bi, 128)], dw3_sl,
                    start=True, stop=False, perf_mode=DR,
                )
                nc.tensor.matmul(ps[:], ones_sl, db3_sl, start=False,
                                 stop=True, perf_mode=DR)
                nc.scalar.activation(st_["ob"][blk][:, bi, :], ps[:],
                                     AF.Sigmoid)

            def store(blk, h):
                # Half-block HWDGE stores, emitted right after the gating
                # sigmoid pair so the transfers overlap remaining compute.
                ob = st_["ob"][blk]
                nc.sync.dma_start(out=out_v[blk, :, 2 * h : 2 * h + 2],
                                  in_=ob[:, 2 * h : 2 * h + 2])

            # Software-pipelined emission: per-engine streams execute in
            # program order, so blk1's L1 is interleaved into blk0's
            # mid-layer chain, and the two blocks' L6/sigmoid groups are
            # interleaved so the final store's gating sigmoid runs early.
            # The first ACT op has no data deps: it absorbs the one-time
            # activation-table load (~1.3us) before real work arrives.
            # (DVE memset first so the source is initialized for CoreSim.)
            nc.vector.memset(wdump[0:1, 0:2], 0.0)
            _stouch(nc, sdump, wdump[0:1, 0:1])
            _touch(nc, scratch, xts[0][:, 0, 0, :])
            a8v = a_sb[:, 0:2560].bitcast(F8)
            _touch(nc, scratch, a8v)
            _vtouch(nc, vdump, a8v[:, 0:1])
            _stouch(nc, sdump, a8v[:, 0:1])
            for gj in range(4):
                l1(0, gj)
            _touch(nc, scratch, xts[1][:, 0, 0, :])
            for gj in range(4):
                l1(1, gj)
            touch_h1(0)
            l2(0)
            l3(0)
            l4(0)
            l5(0, 0)
            l5(0, 1)
            touch_h1(1)
            l2(1)
            l3(1)
            l4(1)
            l5(1, 0)
            l5(1, 1)
            _touch(nc, scratch, w8_sb[:, 0, :])
            _touch(nc, scratch, st_["d2"][0][:, 1, :])
            l6(0, 0)
            l6(0, 1)
            store(0, 0)
            _touch(nc, scratch, st_["d2"][1][:, 1, :])
            l6(1, 0)
            l6(1, 1)
            store(1, 0)
            l6(0, 2)
            l6(0, 3)
            store(0, 1)
            l6(1, 2)
            l6(1, 3)
            store(1, 1)

    return nc


def _get_program():
    if "prog" not in _CACHE:
        _CACHE["prog"] = _build_program()
    return _CACHE["prog"]


def _to_bf16(a):
    """Round-to-nearest-even fp32 -> bf16, returned as a uint16 view."""
    u = np.ascontiguousarray(a, dtype=np.float32).view(np.uint32)
    return (((u >> 16) + ((u >> 15) & 1)) & 0xFFFF).astype(np.uint16)


def _bf16_arr(u16):
    import ml_dtypes

    return u16.view(ml_dtypes.bfloat16)


def _f8(a):
    import ml_dtypes

    return np.asarray(a, dtype=np.float32).astype(ml_dtypes.float8_e4m3)


def _put_f32(pack_u16, rows, col2, vals):
    """Embed fp32 values as adjacent uint16 pairs at bf16 column col2."""
    v = np.asarray(vals, dtype=np.float32).view(np.uint32)
    pack_u16[:rows, col2] = (v & 0xFFFF).astype(np.uint16)
    pack_u16[:rows, col2 + 1] = (v >> 16).astype(np.uint16)


def _pack_params(inputs):
    import ml_dtypes

    gw = np.asarray(inputs["gw"], dtype=np.float32)
    gb = np.asarray(inputs["gb"], dtype=np.float32)
    idx = np.asarray(inputs["idx"], dtype=np.int64)
    mask = np.asarray(inputs["mask"], dtype=np.float32)
    w1 = np.asarray(inputs["w1"], dtype=np.float32)
    b1 = np.asarray(inputs["b1"], dtype=np.float32)
    w2 = np.asarray(inputs["w2"], dtype=np.float32)
    b2 = np.asarray(inputs["b2"], dtype=np.float32)
    dw1 = np.asarray(inputs["dw1"], dtype=np.float32)
    db1 = np.asarray(inputs["db1"], dtype=np.float32)
    dw2 = np.asarray(inputs["dw2"], dtype=np.float32)
    db2 = np.asarray(inputs["db2"], dtype=np.float32)
    dw3 = np.asarray(inputs["dw3"], dtype=np.float32)
    db3 = np.asarray(inputs["db3"], dtype=np.float32)

    g, k = idx.shape
    assert g == G

    # Fold gather + grouped Dense(1) into a dense [S, GP] matrix.
    a_mat = np.zeros((S, GP), dtype=np.float32)
    gwm = (gw * mask).astype(np.float32)
    cols = np.repeat(np.arange(g, dtype=np.int64), k)
    np.add.at(a_mat, (idx.reshape(-1), cols), gwm.reshape(-1))

    # apack raw bytes [p, APR_COLS]: A+w1 fp8 in DoubleRow layout ([P, i,
    # gj|4=w1, m] with rows s/g = P*256 + i*128 + p), then fp32 biases and
    # bf16 mid weights as raw bytes.
    apack = np.zeros((128, APR_COLS), dtype=np.uint8)
    a8 = _f8(a_mat).reshape(2, 2, 128, 4, 128)        # [P, i, p, gj, m]
    w1_pad = np.zeros((GP, HID), dtype=np.float32)
    w1_pad[:g] = w1
    w18 = _f8(w1_pad).reshape(2, 2, 128, 128)         # [P, i, p, m]
    a_full = np.zeros((128, 2, 2, 5, 128), dtype=ml_dtypes.float8_e4m3)
    a_full[:, :, :, 0:4, :] = a8.transpose(2, 0, 1, 3, 4)
    a_full[:, :, :, 4, :] = w18.transpose(2, 0, 1, 3)
    apack[:, A_BASE : A_BASE + 2560] = \
        a_full.reshape(128, 2560).view(np.uint8)

    def put_f32_bytes(col, vals, rows=128):
        v = np.asarray(vals, dtype=np.float32)
        apack[:rows, col : col + 4] = v.view(np.uint8).reshape(rows, 4)

    gb_pad = np.zeros(GP, np.float32)
    gb_pad[:g] = gb
    for i in range(4):
        put_f32_bytes(BIAS_BASE + 4 * i, gb_pad[i * 128 : (i + 1) * 128])
    put_f32_bytes(BIAS_BASE + 16, b1)
    put_f32_bytes(BIAS_BASE + 20, b2, rows=LAT)
    put_f32_bytes(BIAS_BASE + 24, db1)
    put_f32_bytes(BIAS_BASE + 28, db2[:128])
    put_f32_bytes(BIAS_BASE + 32, db2[128:])
    apack[:, W2_BASE : W2_BASE + 2 * LAT] = \
        _to_bf16(w2).view(np.uint8).reshape(128, 2 * LAT)
    apack[:LAT, DW1_BASE : DW1_BASE + 2 * HID] = \
        _to_bf16(dw1).view(np.uint8).reshape(LAT, 2 * HID)
    apack[:, DW2_BASE : DW2_BASE + 512] = \
        _to_bf16(dw2).view(np.uint8).reshape(128, 512)

    # w8pack [p, i, free]: dw3 rows k = i*128 + p; ones + compensated db3
    # rows on partition 0.
    w8 = np.zeros((128, 2, W8_FREE), dtype=ml_dtypes.float8_e4m3)
    dw38 = _f8(dw3).reshape(2, 128, S)                # [i, p, s]
    w8[:, :, DW3_OFF : DW3_OFF + S] = dw38.transpose(1, 0, 2)
    w8[0, :, ONES_OFF : ONES_OFF + 128] = np.float32(1.0)
    db3_hi = _f8(db3)
    db3_lo = _f8(db3 - db3_hi.astype(np.float32))
    w8[0, 0, DB3_OFF : DB3_OFF + S] = db3_hi
    w8[0, 1, DB3_OFF : DB3_OFF + S] = db3_lo

    return apack, w8


def kernel(**inputs) -> np.ndarray:
    global last_results

    x = np.asarray(inputs["x"], dtype=np.float32)
    assert x.shape == (B, S), x.shape
    x8 = _f8(x)
    apack, w8pack = _pack_params(inputs)

    in_maps = [
        {"xt": np.ascontiguousarray(x8[c * BC : (c + 1) * BC].T),
         "apack": apack, "w8pack": w8pack}
        for c in range(NCORES)
    ]

    nc = _get_program()
    trace = os.environ.get("KERNEL_TRACE", "0") == "1"
    res = run_bass_kernel_spmd(nc, in_maps, list(range(NCORES)), trace=trace)
    last_results = res
    out = np.concatenate(
        [np.asarray(r["out"]) for r in res.results], axis=0
    )
    return out.astype(np.float32)


if __name__ == "__main__":
    rng = np.random.RandomState(0)
    demo = {
        "x": rng.rand(B, S).astype(np.float32),
        "gw": rng.randn(G, 30).astype(np.float32) * 0.18,
        "gb": rng.randn(G).astype(np.float32) * 0.1,
        "idx": rng.randint(0, S, (G, 30)).astype(np.int32),
        "mask": (rng.rand(G, 30) > 0.5).astype(np.float32),
        "w1": rng.randn(G, HID).astype(np.float32) * 0.04,
        "b1": rng.randn(HID).astype(np.float32) * 0.1,
        "w2": rng.randn(HID, LAT).astype(np.float32) * 0.09,
        "b2": rng.randn(LAT).astype(np.float32) * 0.1,
        "dw1": rng.randn(LAT, HID).astype(np.float32) * 0.18,
        "db1": rng.randn(HID).astype(np.float32) * 0.1,
        "dw2": rng.randn(HID, HID2).astype(np.float32) * 0.09,
        "db2": rng.randn(HID2).astype(np.float32) * 0.1,
        "dw3": rng.randn(HID2, S).astype(np.float32) * 0.06,
        "db3": rng.randn(S).astype(np.float32) * 0.1,
    }
    out = kernel(**demo)

    # numpy check of the dense-folded network
    x = demo["x"].astype(np.float64)
    a_mat = np.zeros((S, GP))
    gwm = demo["gw"] * demo["mask"]
    cols = np.repeat(np.arange(G), 30)
    np.add.at(a_mat, (demo["idx"].reshape(-1).astype(np.int64), cols),
              gwm.reshape(-1).astype(np.float64))
    gb_pad = np.zeros(GP)
    gb_pad[:G] = demo["gb"]
    w1_pad = np.zeros((GP, HID))
    w1_pad[:G] = demo["w1"]
    relu = lambda v: np.maximum(v, 0)
    h = relu(x @ a_mat + gb_pad)
    h = relu(h @ w1_pad + demo["b1"])
    z = relu(h @ demo["w2"] + demo["b2"])
    d = relu(z @ demo["dw1"] + demo["db1"])
    d = relu(d @ demo["dw2"] + demo["db2"])
    exp = 1 / (1 + np.exp(-(d @ demo["dw3"] + demo["db3"])))
    err = np.linalg.norm(out - exp) / np.linalg.norm(exp)
    print("out", out.shape, out.dtype, float(out.mean()))
    print("demo rel_fro vs numpy:", err)


# revision 60
# speedup vs baseline: 1.1712x; 1.1347x over previous
"""Trainium2 Bass kernel for the CustomAutoencoder problem.

Network (per batch row):
    h  = relu(einsum('gk,k->g', gw*mask, x[idx]) + gb)   # grouped gather-dense
    h  = relu(h @ w1 + b1); z = relu(h @ w2 + b2)
    d  = relu(z @ dw1 + db1); d = relu(d @ dw2 + db2)
    out = sigmoid(d @ dw3 + db3)

The gather+grouped-dense encoder is mathematically x @ A with
A[s, g] = sum_k (gw*mask)[g, k] * (idx[g, k] == s), so the whole model is a
dense MLP chain.  A is built on the host from the small param tensors
(replicated per the data-parallel sharding) and the batch dim is sharded
across 8 NeuronCores.

Datapath: x is transposed AND cast to fp8e4m3 on the host, so the device
runs a pure matmul chain with no on-device transposes.  The three big
matmuls (L1 x@A, L2 h1@w1, L6 d2@dw3) run as fp8 DoubleRow (2 contraction
rows per partition, 0.5 PE cycles/output-row); mid layers stay bf16 and all
PSUM accumulation is fp32 (measured end-to-end rel_fro ~2e-3, gate 2e-2).
db3 enters the L6 accumulation group as a ones-outer-product DoubleRow
matmul with a two-term fp8 compensation (hi + residual).  The output is
stored bf16 and upconverted on the host.

PSUM is only readable by DVE and ACT on TRN2 (GPSIMD/Pool cannot access
PSUM - BIR verifier), so the PSUM drains are balanced across those two:
DVE takes h1 pair0 / h2 / z / d2, ACT takes h1 pair1 / d1 / the sigmoids.
Each drained tile is written by exactly one engine so slice writes never
mix engines.

DoubleRow operand layout (verified in CoreSim): lhsT [p, 2, M], rhs
[p, 2, N], out[M, N] = sum_i lhsT[:, i, :].T @ rhs[:, i, :], i.e. logical
contraction k = i*P + p.

DMA: 4 loads + 4 half-block stores fit in the 8 HWDGE queues so no
queue-order wait is needed; each store is emitted right after its gating
sigmoid pair so the transfers overlap the remaining compute.  The tiny
"touch" matmuls / copies pre-advance each engine's observed vector clock
past DMA producers: walrus allows only ONE sync wait on matmul (S3_LW),
activation (S3D3_AC) and DMA instructions, so every real instruction must
need at most one semaphore wait.
"""

import os
import sys

sys.path.insert(0, "/opt/trn_rl_repo")

import numpy as np

import concourse.bass as bass
import concourse.tile as tile
from concourse import mybir
from concourse.bass import ts
from concourse.bass_utils import run_bass_kernel_spmd

F32 = mybir.dt.float32
BF = mybir.dt.bfloat16
F8 = mybir.dt.float8e4
U8 = mybir.dt.uint8
AF = mybir.ActivationFunctionType
DR = mybir.MatmulPerfMode.DoubleRow

B = 8192          # full batch
S = 512           # sample size (input/output features)
G = 510           # number of groups
GP = 512          # G padded to a multiple of 128
HID = 128
LAT = 32
HID2 = 256
NCORES = 8
BC = B // NCORES  # rows per core
BBLK = 512        # batch columns per block (PSUM free-dim max for fp32)
NBLK = BC // BBLK

# apack (fp8 bytes) [128, APR_COLS]: A+w1 in DoubleRow layout, then the
# fp32 biases and bf16 mid-layer weights embedded as raw bytes (the device
# reads them through bitcast slices).  One DMA covers everything the front
# half of the network needs.
A_BASE = 0            # 2560 cols: [P(2), i(2), gj(5), m(128)], gj=4 is w1
BIAS_BASE = 2560      # 9 fp32 bias cols (4 bytes each): +0..3 gb chunks,
                      # +4 b1, +5 b2 (rows<32), +6 db1, +7..8 db2 chunks
W2_BASE = 2596        # 128 x 32 bf16 (64 cols)
DW1_BASE = 2660       # 32 x 128 bf16 (256 cols, rows 0-31)
DW2_BASE = 2916       # 128 x 256 bf16 (512 cols)
APR_COLS = 3428

# w8pack (fp8) [128, 2, W8_FREE]: dim1 is the DoubleRow k-tile.
DW3_OFF = 0           # [:, i, 0:512]   dw3[i*128+p, s]
ONES_OFF = 512        # [0, i, 512:640] all-ones rows
DB3_OFF = 640         # [0, 0, :] = fp8(db3), [0, 1, :] = fp8(db3 - fp8(db3))
W8_FREE = 1152

_CACHE: dict = {}
last_results = None


def _touch(nc, scratch, tl):
    """1x1 PE matmul reading a corner of `tl`: advances the PE engine's
    observed vector clock past tl's producer (walrus S3_LW single-wait)."""
    return nc.tensor.matmul(
        scratch[0:1, 0:2], tl[0:1, 0:1], tl[0:1, 0:2], start=True, stop=True
    )


_DUMP_IDX = [0, 0, 0]


def _stouch(nc, sdump, tl):
    """Scalar-engine equivalent of _touch for ACT-consumed (bias) tiles."""
    k = _DUMP_IDX[0] % 32
    _DUMP_IDX[0] += 1
    return nc.scalar.copy(out=sdump[0:1, k : k + 1], in_=tl[0:1, 0:1])


def _vtouch(nc, vdump, tl):
    """Vector-engine equivalent of _touch for DVE-consumed tiles."""
    k = _DUMP_IDX[1] % 32
    _DUMP_IDX[1] += 1
    return nc.vector.tensor_copy(vdump[0:1, k : k + 1], tl[0:1, 0:1])


class SplitDrainTileContext(tile.TileContext):
    """TileContext whose kernel-tail drain carries at most one sync wait per
    instruction: this walrus build rejects >1 sync wait on any instruction,
    and the stock tail drain aggregates one wait per active proc."""

    def _drain_and_barrier(self, tick_clock, wait_clock):
        from concourse.vector_clock import ScopedClock, VectorClock

        gc = tick_clock.global_clock
        n = len(gc)
        for p in range(n):
            t = gc[p]
            if t == 0:
                continue
            single = [0] * n
            single[p] = t
            nop = self.nc.sync.nop(nofuse=True, hint="split_drain_wait")
            wait_clock.add_sem_waits(
                nop.ins, ScopedClock({None: VectorClock(single)})
            )
        # The per-proc nops above already enforce every outstanding tick in
        # SP program order, so the drain itself needs no waits.
        self.nc.sync.drain()
        self.nc.all_engine_barrier()
        assert self.sems is not None
        popped = self.nc._tile_sem_poison_stack.pop()
        assert popped is self._sem_poison
        self.nc.clear_and_free_semaphores(list(self.sems.allocated().values()))
        self.nc.all_engine_barrier()


def _build_program():
    nc = bass.Bass()

    # x arrives host-transposed [S, BC] fp8: row s = P*256 + i*128 + p.
    xt_d = nc.declare_dram_parameter("xt", [S, BC], F8, isOutput=False)
    a_d = nc.declare_dram_parameter("apack", [128, APR_COLS], U8,
                                    isOutput=False)
    w8_d = nc.declare_dram_parameter("w8pack", [128, 2, W8_FREE], F8,
                                     isOutput=False)
    out_d = nc.declare_dram_parameter("out", [BC, S], BF, isOutput=True)

    xt_v = xt_d.rearrange("(P i p) (k b) -> k p P i b", P=2, i=2, p=128,
                          b=BBLK)                       # [NBLK,128,2,2,512]
    out_v = out_d.rearrange("(k i p) s -> k p i s", p=128, i=4)

    with SplitDrainTileContext(nc) as tc:
        with (
            tc.tile_pool(name="weights", bufs=1) as wp,
            tc.tile_pool(name="xin", bufs=2) as xp,
            tc.tile_pool(name="h1", bufs=4) as h1p,
            tc.tile_pool(name="acts", bufs=4) as ap_,
            tc.tile_pool(name="d2", bufs=2) as d2p,
            tc.tile_pool(name="outs", bufs=2) as op_,
            tc.tile_pool(name="p1", bufs=3, space="PSUM") as p1p,
            tc.tile_pool(name="pmid", bufs=2, space="PSUM") as pmp,
            tc.tile_pool(name="p6", bufs=2, space="PSUM") as p6p,
            tc.tile_pool(name="psc", bufs=1, space="PSUM") as pscp,
        ):
            scratch = pscp.tile([1, 2], F32)
            sdump = wp.tile([1, 32], F32, tag="sdump")
            vdump = wp.tile([1, 32], F32, tag="vdump")
            wdump = wp.tile([1, 2], F32, tag="wdump")
            _DUMP_IDX[0] = _DUMP_IDX[1] = _DUMP_IDX[2] = 0

            # Loads in need order; SP serializes transfers in issue order.
            xts = []
            xt = xp.tile([128, 2, 2, BBLK], F8, tag="xt")
            nc.sync.dma_start(out=xt[:], in_=xt_v[0])
            xts.append(xt)
            a_sb = wp.tile([128, APR_COLS], U8, tag="apack")
            nc.sync.dma_start(out=a_sb[:], in_=a_d[:, :])
            xt = xp.tile([128, 2, 2, BBLK], F8, tag="xt")
            nc.sync.dma_start(out=xt[:], in_=xt_v[1])
            xts.append(xt)
            w8_sb = wp.tile([128, 2, W8_FREE], F8, tag="w8pack")
            nc.sync.dma_start(out=w8_sb[:], in_=w8_d[:])

            a5 = a_sb[:, A_BASE : A_BASE + 2560].bitcast(F8).rearrange(
                "p (P i g m) -> p P i g m", P=2, i=2, g=5, m=128
            )

            def a_sl(P, gj):         # lhsT [128, 2, 128], DR pair P
                return a5[:, P, :, gj, :]

            def w1_sl(P):            # lhsT [128, 2, 128]
                return a5[:, P, :, 4, :]

            w2_sl = a_sb[:, W2_BASE : W2_BASE + 2 * LAT].bitcast(BF)
            dw1_sl = a_sb[0:LAT, DW1_BASE : DW1_BASE + 2 * HID].bitcast(BF)

            def dw2_sl(j):           # lhsT [128, 128]
                c = DW2_BASE + j * 256
                return a_sb[:, c : c + 256].bitcast(BF)

            dw3_sl = w8_sb[:, :, DW3_OFF : DW3_OFF + S]       # [128, 2, 512]
            ones_sl = w8_sb[0:1, :, ONES_OFF : ONES_OFF + 128]  # [1, 2, 128]
            db3_sl = w8_sb[0:1, :, DB3_OFF : DB3_OFF + S]     # [1, 2, 512]

            def bias_col(i, rows=128):
                c = BIAS_BASE + 4 * i
                return a_sb[0:rows, c : c + 4].bitcast(F32)

            gb_b = [bias_col(i) for i in range(4)]
            b1_b = bias_col(4)
            b2_b = bias_col(5, rows=LAT)
            db1_b = bias_col(6)
            db2_b = [bias_col(7 + j) for j in range(2)]

            st_ = {"h1": {}, "h2": {}, "z": {}, "d1": {}, "d2": {}, "ob": {}}

            def relu_drain(eng, out_ap, ps, bias):
                """PSUM -> SBUF bias+relu on the chosen engine."""
                if eng == "dve":
                    nc.vector.tensor_scalar(
                        out_ap, ps[:], bias, 0.0,
                        op0=mybir.AluOpType.add, op1=mybir.AluOpType.max,
                    )
                else:
                    nc.scalar.activation(out_ap, ps[:], AF.Relu, bias=bias)

            # Drain-engine assignment: each drained tile is written by one
            # engine.  ACT carries the 8 sigmoids plus the h1 pair-1 tiles
            # and d1; DVE the rest.
            H1_ENG = {(0, 0): "dve", (0, 1): "act",
                      (1, 0): "dve", (1, 1): "act"}
            Z_ENG = {0: "dve", 1: "dve"}

            def l1(blk, gj):
                # h1T[gj] = relu(sum_s A[s, gj].T x[b, s] + gb[gj]), fp8 DR.
                ps = p1p.tile([128, BBLK], F32, tag="p1")
                for P in range(2):
                    nc.tensor.matmul(
                        ps[:], a_sl(P, gj), xts[blk][:, P, :, :],
                        start=(P == 0), stop=(P == 1), perf_mode=DR,
                    )
                if gj == 0:
                    h = h1p.tile([128, 2, BBLK], F8, tag="h1a")
                    st_["h1"].setdefault(blk, []).append(h)
                elif gj == 2:
                    h = h1p.tile([128, 2, BBLK], F8, tag="h1b")
                    st_["h1"][blk].append(h)
                h = st_["h1"][blk][gj // 2]
                relu_drain(H1_ENG[(blk, gj // 2)], h[:, gj % 2, :], ps,
                           gb_b[gj])

            def touch_h1(blk):
                # PE observes the producing engines' ticks of both h1 pair
                # tiles, so L2 matmuls keep at most one sync wait.
                for h in st_["h1"][blk]:
                    _touch(nc, scratch, h[:, 1, :])

            def l2(blk):
                pp = p6p if blk == 1 else pmp
                ps = pp.tile([128, BBLK], F32,
                             tag="p6" if blk == 1 else "pmid")
                for P in range(2):
                    nc.tensor.matmul(
                        ps[:], w1_sl(P), st_["h1"][blk][P][:],
                        start=(P == 0), stop=(P == 1), perf_mode=DR,
                    )
                h2 = ap_.tile([HID, BBLK], BF, tag="h2")
                relu_drain("dve", h2[:], ps, b1_b)
                st_["h2"][blk] = h2

            def l3(blk):
                pp = p6p if blk == 1 else pmp
                ps = pp.tile([LAT, BBLK], F32,
                             tag="p6" if blk == 1 else "pmid")
                nc.tensor.matmul(ps[:], w2_sl, st_["h2"][blk][:], start=True,
                                 stop=True)
                z = ap_.tile([LAT, BBLK], BF, tag="z")
                relu_drain(Z_ENG[blk], z[:], ps, b2_b)
                st_["z"][blk] = z

            def l4(blk):
                pp = p6p if blk == 1 else pmp
                ps = pp.tile([HID, BBLK], F32,
                             tag="p6" if blk == 1 else "pmid")
                nc.tensor.matmul(ps[:], dw1_sl, st_["z"][blk][:], start=True,
                                 stop=True)
                d1 = ap_.tile([HID, BBLK], BF, tag="d1")
                relu_drain("act", d1[:], ps, db1_b)
                st_["d1"][blk] = d1
                # PE observes the ACT tick of d1 so L5 matmuls keep at most
                # one sync wait (their psum-slot wait).
                _touch(nc, scratch, d1)

            def l5(blk, j):
                pp = p6p if blk == 1 else pmp
                ps = pp.tile([128, BBLK], F32,
                             tag="p6" if blk == 1 else "pmid")
                nc.tensor.matmul(ps[:], dw2_sl(j), st_["d1"][blk][:],
                                 start=True, stop=True)
                if j == 0:
                    d2 = d2p.tile([128, 2, BBLK], F8, tag="d2")
                    st_["d2"][blk] = d2
                d2 = st_["d2"][blk]
                relu_drain("dve", d2[:, j, :], ps, db2_b[j])

            def l6(blk, bi, pool=None):
                # out[bi] = sigmoid(d2T[:, bi].T @ dw3 + db3), natural
                # layout; both the data matmul and the db3 broadcast
                # (ones-outer-product, hi+residual fp8 compensation) are
                # fp8 DoubleRow in one accumulation group.  Late l6 groups
                # borrow the (by then idle) pmid banks so four sigmoid
                # psums are in flight at once.
                if bi == 0:
                    ob = op_.tile([128, 4, S], BF, tag="ob")
                    st_["ob"][blk] = ob
                ps = (pool or (p6p if blk == 1 else pmp)).tile(
                    [128, S], F32, tag="p6" if blk == 1 else "pmid")
                nc.tensor.matmul(
                    ps[:], st_["d2"][blk][:, :, ts(bi, 128)], dw3_sl,
                    start=True, stop=False, perf_mode=DR,
                )
                nc.tensor.matmul(ps[:], ones_sl, db3_sl, start=False,
                                 stop=True, perf_mode=DR)
                nc.scalar.activation(st_["ob"][blk][:, bi, :], ps[:],
                                     AF.Sigmoid)

            def store(blk, h):
                # Half-block HWDGE stores, emitted right after the gating
                # sigmoid pair so the transfers overlap remaining compute.
                ob = st_["ob"][blk]
                nc.sync.dma_start(out=out_v[blk, :, 2 * h : 2 * h + 2],
                                  in_=ob[:, 2 * h : 2 * h + 2])

            # Software-pipelined emission: per-engine streams execute in
            # program order, so blk1's L1 is interleaved into blk0's
            # mid-layer chain, and the two blocks' L6/sigmoid groups are
            # interleaved so the final store's gating sigmoid runs early.
            # The first ACT op has no data deps: it absorbs the one-time
            # activation-table load (~1.3us) before real work arrives.
            # (DVE memset first so the source is initialized for CoreSim.)
            nc.vector.memset(wdump[0:1, 0:2], 0.0)
            _stouch(nc, sdump, wdump[0:1, 0:1])
            _touch(nc, scratch, xts[0][:, 0, 0, :])
            a8v = a_sb[:, 0:2560].bitcast(F8)
            _touch(nc, scratch, a8v)
            _vtouch(nc, vdump, a8v[:, 0:1])
            _stouch(nc, sdump, a8v[:, 0:1])
            for gj in range(4):
                l1(0, gj)
            _touch(nc, scratch, xts[1][:, 0, 0, :])
            for gj in range(4):
                l1(1, gj)
            touch_h1(0)
            l2(0)
            l3(0)
            l4(0)
            l5(0, 0)
            l5(0, 1)
            touch_h1(1)
            l2(1)
            l3(1)
            l4(1)
            l5(1, 0)
            l5(1, 1)
            _touch(nc, scratch, w8_sb[:, 0, :])
            _touch(nc, scratch, st_["d2"][0][:, 1, :])
            l6(0, 0)
            l6(0, 1)
            store(0, 0)
            _touch(nc, scratch, st_["d2"][1][:, 1, :])
            l6(1, 0)
            l6(1, 1)
            store(1, 0)
            l6(0, 2)
            l6(0, 3)
            store(0, 1)
            l6(1, 2)
            l6(1, 3)
            store(1, 1)

    return nc


def _get_program():
    if "prog" not in _CACHE:
        _CACHE["prog"] = _build_program()
    return _CACHE["prog"]


def _to_bf16(a):
    """Round-to-nearest-even fp32 -> bf16, returned as a uint16 view."""
    u = np.ascontiguousarray(a, dtype=np.float32).view(np.uint32)
    return (((u >> 16) + ((u >> 15) & 1)) & 0xFFFF).astype(np.uint16)


def _bf16_arr(u16):
    import ml_dtypes

    return u16.view(ml_dtypes.bfloat16)


def _f8(a):
    import ml_dtypes

    return np.asarray(a, dtype=np.float32).astype(ml_dtypes.float8_e4m3)


def _put_f32(pack_u16, rows, col2, vals):
    """Embed fp32 values as adjacent uint16 pairs at bf16 column col2."""
    v = np.asarray(vals, dtype=np.float32).view(np.uint32)
    pack_u16[:rows, col2] = (v & 0xFFFF).astype(np.uint16)
    pack_u16[:rows, col2 + 1] = (v >> 16).astype(np.uint16)


def _pack_params(inputs):
    import ml_dtypes

    gw = np.asarray(inputs["gw"], dtype=np.float32)
    gb = np.asarray(inputs["gb"], dtype=np.float32)
    idx = np.asarray(inputs["idx"], dtype=np.int64)
    mask = np.asarray(inputs["mask"], dtype=np.float32)
    w1 = np.asarray(inputs["w1"], dtype=np.float32)
    b1 = np.asarray(inputs["b1"], dtype=np.float32)
    w2 = np.asarray(inputs["w2"], dtype=np.float32)
    b2 = np.asarray(inputs["b2"], dtype=np.float32)
    dw1 = np.asarray(inputs["dw1"], dtype=np.float32)
    db1 = np.asarray(inputs["db1"], dtype=np.float32)
    dw2 = np.asarray(inputs["dw2"], dtype=np.float32)
    db2 = np.asarray(inputs["db2"], dtype=np.float32)
    dw3 = np.asarray(inputs["dw3"], dtype=np.float32)
    db3 = np.asarray(inputs["db3"], dtype=np.float32)

    g, k = idx.shape
    assert g == G

    # Fold gather + grouped Dense(1) into a dense [S, GP] matrix.
    a_mat = np.zeros((S, GP), dtype=np.float32)
    gwm = (gw * mask).astype(np.float32)
    cols = np.repeat(np.arange(g, dtype=np.int64), k)
    np.add.at(a_mat, (idx.reshape(-1), cols), gwm.reshape(-1))

    # apack raw bytes [p, APR_COLS]: A+w1 fp8 in DoubleRow layout ([P, i,
    # gj|4=w1, m] with rows s/g = P*256 + i*128 + p), then fp32 biases and
    # bf16 mid weights as raw bytes.
    apack = np.zeros((128, APR_COLS), dtype=np.uint8)
    a8 = _f8(a_mat).reshape(2, 2, 128, 4, 128)        # [P, i, p, gj, m]
    w1_pad = np.zeros((GP, HID), dtype=np.float32)
    w1_pad[:g] = w1
    w18 = _f8(w1_pad).reshape(2, 2, 128, 128)         # [P, i, p, m]
    a_full = np.zeros((128, 2, 2, 5, 128), dtype=ml_dtypes.float8_e4m3)
    a_full[:, :, :, 0:4, :] = a8.transpose(2, 0, 1, 3, 4)
    a_full[:, :, :, 4, :] = w18.transpose(2, 0, 1, 3)
    apack[:, A_BASE : A_BASE + 2560] = \
        a_full.reshape(128, 2560).view(np.uint8)

    def put_f32_bytes(col, vals, rows=128):
        v = np.asarray(vals, dtype=np.float32)
        apack[:rows, col : col + 4] = v.view(np.uint8).reshape(rows, 4)

    gb_pad = np.zeros(GP, np.float32)
    gb_pad[:g] = gb
    for i in range(4):
        put_f32_bytes(BIAS_BASE + 4 * i, gb_pad[i * 128 : (i + 1) * 128])
    put_f32_bytes(BIAS_BASE + 16, b1)
    put_f32_bytes(BIAS_BASE + 20, b2, rows=LAT)
    put_f32_bytes(BIAS_BASE + 24, db1)
    put_f32_bytes(BIAS_BASE + 28, db2[:128])
    put_f32_bytes(BIAS_BASE + 32, db2[128:])
    apack[:, W2_BASE : W2_BASE + 2 * LAT] = \
        _to_bf16(w2).view(np.uint8).reshape(128, 2 * LAT)
    apack[:LAT, DW1_BASE : DW1_BASE + 2 * HID] = \
        _to_bf16(dw1).view(np.uint8).reshape(LAT, 2 * HID)
    apack[:, DW2_BASE : DW2_BASE + 512] = \
        _to_bf16(dw2).view(np.uint8).reshape(128, 512)

    # w8pack [p, i, free]: dw3 rows k = i*128 + p; ones + compensated db3
    # rows on partition 0.
    w8 = np.zeros((128, 2, W8_FREE), dtype=ml_dtypes.float8_e4m3)
    dw38 = _f8(dw3).reshape(2, 128, S)                # [i, p, s]
    w8[:, :, DW3_OFF : DW3_OFF + S] = dw38.transpose(1, 0, 2)
    w8[0, :, ONES_OFF : ONES_OFF + 128] = np.float32(1.0)
    db3_hi = _f8(db3)
    db3_lo = _f8(db3 - db3_hi.astype(np.float32))
    w8[0, 0, DB3_OFF : DB3_OFF + S] = db3_hi
    w8[0, 1, DB3_OFF : DB3_OFF + S] = db3_lo

    return apack, w8


def kernel(**inputs) -> np.ndarray:
    global last_results

    x = np.asarray(inputs["x"], dtype=np.float32)
    assert x.shape == (B, S), x.shape
    x8 = _f8(x)
    apack, w8pack = _pack_params(inputs)

    in_maps = [
        {"xt": np.ascontiguousarray(x8[c * BC : (c + 1) * BC].T),
         "apack": apack, "w8pack": w8pack}
        for c in range(NCORES)
    ]

    nc = _get_program()
    trace = os.environ.get("KERNEL_TRACE", "0") == "1"
    res = run_bass_kernel_spmd(nc, in_maps, list(range(NCORES)), trace=trace)
    last_results = res
    out = np.concatenate(
        [np.asarray(r["out"]) for r in res.results], axis=0
    )
    return out.astype(np.float32)


if __name__ == "__main__":
    rng = np.random.RandomState(0)
    demo = {
        "x": rng.rand(B, S).astype(np.float32),
        "gw": rng.randn(G, 30).astype(np.float32) * 0.18,
        "gb": rng.randn(G).astype(np.float32) * 0.1,
        "idx": rng.randint(0, S, (G, 30)).astype(np.int32),
        "mask": (rng.rand(G, 30) > 0.5).astype(np.float32),
        "w1": rng.randn(G, HID).astype(np.float32) * 0.04,
        "b1": rng.randn(HID).astype(np.float32) * 0.1,
        "w2": rng.randn(HID, LAT).astype(np.float32) * 0.09,
        "b2": rng.randn(LAT).astype(np.float32) * 0.1,
        "dw1": rng.randn(LAT, HID).astype(np.float32) * 0.18,
        "db1": rng.randn(HID).astype(np.float32) * 0.1,
        "dw2": rng.randn(HID, HID2).astype(np.float32) * 0.09,
        "db2": rng.randn(HID2).astype(np.float32) * 0.1,
        "dw3": rng.randn(HID2, S).astype(np.float32) * 0.06,
        "db3": rng.randn(S).astype(np.float32) * 0.1,
    }
    out = kernel(**demo)

    # numpy check of the dense-folded network
    x = demo["x"].astype(np.float64)
    a_mat = np.zeros((S, GP))
    gwm = demo["gw"] * demo["mask"]
    cols = np.repeat(np.arange(G), 30)
    np.add.at(a_mat, (demo["idx"].reshape(-1).astype(np.int64), cols),
              gwm.reshape(-1).astype(np.float64))
    gb_pad = np.zeros(GP)
    gb_pad[:G] = demo["gb"]
    w1_pad = np.zeros((GP, HID))
    w1_pad[:G] = demo["w1"]
    relu = lambda v: np.maximum(v, 0)
    h = relu(x @ a_mat + gb_pad)
    h = relu(h @ w1_pad + demo["b1"])
    z = relu(h @ demo["w2"] + demo["b2"])
    d = relu(z @ demo["dw1"] + demo["db1"])
    d = relu(d @ demo["dw2"] + demo["db2"])
    exp = 1 / (1 + np.exp(-(d @ demo["dw3"] + demo["db3"])))
    err = np.linalg.norm(out - exp) / np.linalg.norm(exp)
    print("out", out.shape, out.dtype, float(out.mean()))
    print("demo rel_fro vs numpy:", err)


# revision 63
# speedup vs baseline: 1.1948x; 1.0201x over previous
"""Trainium2 Bass kernel for the CustomAutoencoder problem.

Network (per batch row):
    h  = relu(einsum('gk,k->g', gw*mask, x[idx]) + gb)   # grouped gather-dense
    h  = relu(h @ w1 + b1); z = relu(h @ w2 + b2)
    d  = relu(z @ dw1 + db1); d = relu(d @ dw2 + db2)
    out = sigmoid(d @ dw3 + db3)

The gather+grouped-dense encoder is mathematically x @ A with
A[s, g] = sum_k (gw*mask)[g, k] * (idx[g, k] == s), so the whole model is a
dense MLP chain.  A is built on the host from the small param tensors
(replicated per the data-parallel sharding) and the batch dim is sharded
across 8 NeuronCores.

Datapath: x is transposed AND cast to fp8e4m3 on the host, so the device
runs a pure matmul chain with no on-device transposes.  The three big
matmuls (L1 x@A, L2 h1@w1, L6 d2@dw3) run as fp8 DoubleRow (2 contraction
rows per partition, 0.5 PE cycles/output-row); mid layers stay bf16 and all
PSUM accumulation is fp32 (measured end-to-end rel_fro ~2e-3, gate 2e-2).
db3 enters the L6 accumulation group as a ones-outer-product DoubleRow
matmul with a two-term fp8 compensation (hi + residual).  The output is
stored bf16 and upconverted on the host.

PSUM is only readable by DVE and ACT on TRN2 (GPSIMD/Pool cannot access
PSUM - BIR verifier), so the PSUM drains are balanced across those two:
DVE takes h1 pair0 / h2 / z / d2, ACT takes h1 pair1 / d1 / the sigmoids.
Each drained tile is written by exactly one engine so slice writes never
mix engines.

DoubleRow operand layout (verified in CoreSim): lhsT [p, 2, M], rhs
[p, 2, N], out[M, N] = sum_i lhsT[:, i, :].T @ rhs[:, i, :], i.e. logical
contraction k = i*P + p.

DMA: 4 loads + 4 half-block stores fit in the 8 HWDGE queues so no
queue-order wait is needed; each store is emitted right after its gating
sigmoid pair so the transfers overlap the remaining compute.  The tiny
"touch" matmuls / copies pre-advance each engine's observed vector clock
past DMA producers: walrus allows only ONE sync wait on matmul (S3_LW),
activation (S3D3_AC) and DMA instructions, so every real instruction must
need at most one semaphore wait.
"""

import os
import sys

sys.path.insert(0, "/opt/trn_rl_repo")

import numpy as np

import concourse.bass as bass
import concourse.tile as tile
from concourse import mybir
from concourse.bass import ts
from concourse.bass_utils import run_bass_kernel_spmd

F32 = mybir.dt.float32
BF = mybir.dt.bfloat16
F8 = mybir.dt.float8e4
U8 = mybir.dt.uint8
AF = mybir.ActivationFunctionType
DR = mybir.MatmulPerfMode.DoubleRow

B = 8192          # full batch
S = 512           # sample size (input/output features)
G = 510           # number of groups
GP = 512          # G padded to a multiple of 128
HID = 128
LAT = 32
HID2 = 256
NCORES = 8
BC = B // NCORES  # rows per core
BBLK = 512        # batch columns per block (PSUM free-dim max for fp32)
NBLK = BC // BBLK

# apack (fp8 bytes) [128, APR_COLS]: A+w1 in DoubleRow layout, then the
# fp32 biases and bf16 mid-layer weights embedded as raw bytes (the device
# reads them through bitcast slices).  One DMA covers everything the front
# half of the network needs.
A_BASE = 0            # 2560 cols: [P(2), i(2), gj(5), m(128)], gj=4 is w1
BIAS_BASE = 2560      # 9 fp32 bias cols (4 bytes each): +0..3 gb chunks,
                      # +4 b1, +5 b2 (rows<32), +6 db1, +7..8 db2 chunks
W2_BASE = 2596        # 128 x 32 bf16 (64 cols)
DW1_BASE = 2660       # 32 x 128 bf16 (256 cols, rows 0-31)
DW2_BASE = 2916       # 128 x 256 bf16 (512 cols)
APR_COLS = 3428

# w8pack (fp8) [128, 2, W8_FREE]: dim1 is the DoubleRow k-tile.
DW3_OFF = 0           # [:, i, 0:512]   dw3[i*128+p, s]
ONES_OFF = 512        # [0, i, 512:640] all-ones rows
DB3_OFF = 640         # [0, 0, :] = fp8(db3), [0, 1, :] = fp8(db3 - fp8(db3))
W8_FREE = 1152

_CACHE: dict = {}
last_results = None


def _touch(nc, scratch, tl):
    """1x1 PE matmul reading a corner of `tl`: advances the PE engine's
    observed vector clock past tl's producer (walrus S3_LW single-wait)."""
    return nc.tensor.matmul(
        scratch[0:1, 0:2], tl[0:1, 0:1], tl[0:1, 0:2], start=True, stop=True
    )


_DUMP_IDX = [0, 0, 0]


def _stouch(nc, sdump, tl):
    """Scalar-engine equivalent of _touch for ACT-consumed (bias) tiles."""
    k = _DUMP_IDX[0] % 32
    _DUMP_IDX[0] += 1
    return nc.scalar.copy(out=sdump[0:1, k : k + 1], in_=tl[0:1, 0:1])


def _vtouch(nc, vdump, tl):
    """Vector-engine equivalent of _touch for DVE-consumed tiles."""
    k = _DUMP_IDX[1] % 32
    _DUMP_IDX[1] += 1
    return nc.vector.tensor_copy(vdump[0:1, k : k + 1], tl[0:1, 0:1])


class SplitDrainTileContext(tile.TileContext):
    """TileContext whose kernel-tail drain carries at most one sync wait per
    instruction: this walrus build rejects >1 sync wait on any instruction,
    and the stock tail drain aggregates one wait per active proc."""

    def _drain_and_barrier(self, tick_clock, wait_clock):
        from concourse.vector_clock import ScopedClock, VectorClock

        gc = tick_clock.global_clock
        n = len(gc)
        for p in range(n):
            t = gc[p]
            if t == 0:
                continue
            single = [0] * n
            single[p] = t
            nop = self.nc.sync.nop(nofuse=True, hint="split_drain_wait")
            wait_clock.add_sem_waits(
                nop.ins, ScopedClock({None: VectorClock(single)})
            )
        # The per-proc nops above already enforce every outstanding tick in
        # SP program order, so the drain itself needs no waits.
        self.nc.sync.drain()
        self.nc.all_engine_barrier()
        assert self.sems is not None
        popped = self.nc._tile_sem_poison_stack.pop()
        assert popped is self._sem_poison
        self.nc.clear_and_free_semaphores(list(self.sems.allocated().values()))
        self.nc.all_engine_barrier()


def _build_program():
    nc = bass.Bass()

    # x arrives host-transposed [S, BC] fp8: row s = P*256 + i*128 + p.
    xt_d = nc.declare_dram_parameter("xt", [S, BC], F8, isOutput=False)
    a_d = nc.declare_dram_parameter("apack", [128, APR_COLS], U8,
                                    isOutput=False)
    w8_d = nc.declare_dram_parameter("w8pack", [128, 2, W8_FREE], F8,
                                     isOutput=False)
    out_d = nc.declare_dram_parameter("out", [BC, S], BF, isOutput=True)

    xt_v = xt_d.rearrange("(P i p) (k b) -> k p P i b", P=2, i=2, p=128,
                          b=BBLK)                       # [NBLK,128,2,2,512]
    out_v = out_d.rearrange("(k i p) s -> k p i s", p=128, i=4)

    with SplitDrainTileContext(nc) as tc:
        with (
            tc.tile_pool(name="weights", bufs=1) as wp,
            tc.tile_pool(name="xin", bufs=2) as xp,
            tc.tile_pool(name="h1", bufs=4) as h1p,
            tc.tile_pool(name="acts", bufs=4) as ap_,
            tc.tile_pool(name="d2", bufs=2) as d2p,
            tc.tile_pool(name="outs", bufs=2) as op_,
            tc.tile_pool(name="p1", bufs=3, space="PSUM") as p1p,
            tc.tile_pool(name="pmid", bufs=2, space="PSUM") as pmp,
            tc.tile_pool(name="p6", bufs=2, space="PSUM") as p6p,
            tc.tile_pool(name="psc", bufs=1, space="PSUM") as pscp,
        ):
            scratch = pscp.tile([1, 2], F32)
            sdump = wp.tile([1, 32], F32, tag="sdump")
            vdump = wp.tile([1, 32], F32, tag="vdump")
            wdump = wp.tile([1, 2], F32, tag="wdump")
            _DUMP_IDX[0] = _DUMP_IDX[1] = _DUMP_IDX[2] = 0

            # Loads in need order; SP serializes transfers in issue order.
            xts = []
            xt = xp.tile([128, 2, 2, BBLK], F8, tag="xt")
            nc.sync.dma_start(out=xt[:], in_=xt_v[0])
            xts.append(xt)
            a_sb = wp.tile([128, APR_COLS], U8, tag="apack")
            nc.sync.dma_start(out=a_sb[:], in_=a_d[:, :])
            xt = xp.tile([128, 2, 2, BBLK], F8, tag="xt")
            nc.sync.dma_start(out=xt[:], in_=xt_v[1])
            xts.append(xt)
            w8_sb = wp.tile([128, 2, W8_FREE], F8, tag="w8pack")
            nc.sync.dma_start(out=w8_sb[:], in_=w8_d[:])

            a5 = a_sb[:, A_BASE : A_BASE + 2560].bitcast(F8).rearrange(
                "p (P i g m) -> p P i g m", P=2, i=2, g=5, m=128
            )

            def a_sl(P, gj):         # lhsT [128, 2, 128], DR pair P
                return a5[:, P, :, gj, :]

            def w1_sl(P):            # lhsT [128, 2, 128]
                return a5[:, P, :, 4, :]

            w2_sl = a_sb[:, W2_BASE : W2_BASE + 2 * LAT].bitcast(BF)
            dw1_sl = a_sb[0:LAT, DW1_BASE : DW1_BASE + 2 * HID].bitcast(BF)

            def dw2_sl(j):           # lhsT [128, 128]
                c = DW2_BASE + j * 256
                return a_sb[:, c : c + 256].bitcast(BF)

            dw3_sl = w8_sb[:, :, DW3_OFF : DW3_OFF + S]       # [128, 2, 512]
            ones_sl = w8_sb[0:1, :, ONES_OFF : ONES_OFF + 128]  # [1, 2, 128]
            db3_sl = w8_sb[0:1, :, DB3_OFF : DB3_OFF + S]     # [1, 2, 512]

            def bias_col(i, rows=128):
                c = BIAS_BASE + 4 * i
                return a_sb[0:rows, c : c + 4].bitcast(F32)

            gb_b = [bias_col(i) for i in range(4)]
            b1_b = bias_col(4)
            b2_b = bias_col(5, rows=LAT)
            db1_b = bias_col(6)
            db2_b = [bias_col(7 + j) for j in range(2)]

            st_ = {"h1": {}, "h2": {}, "z": {}, "d1": {}, "d2": {}, "ob": {}}

            def relu_drain(eng, out_ap, ps, bias):
                """PSUM -> SBUF bias+relu on the chosen engine."""
                if eng == "dve":
                    nc.vector.tensor_scalar(
                        out_ap, ps[:], bias, 0.0,
                        op0=mybir.AluOpType.add, op1=mybir.AluOpType.max,
                    )
                else:
                    nc.scalar.activation(out_ap, ps[:], AF.Relu, bias=bias)

            # Drain-engine assignment: each drained tile is written by one
            # engine.  ACT carries the 8 sigmoids plus the h1 pair-1 tiles
            # and d1; DVE the rest.
            H1_ENG = {(0, 0): "dve", (0, 1): "act",
                      (1, 0): "dve", (1, 1): "act"}
            Z_ENG = {0: "act", 1: "dve"}

            def l1(blk, gj):
                # h1T[gj] = relu(sum_s A[s, gj].T x[b, s] + gb[gj]), fp8 DR.
                ps = p1p.tile([128, BBLK], F32, tag="p1")
                for P in range(2):
                    nc.tensor.matmul(
                        ps[:], a_sl(P, gj), xts[blk][:, P, :, :],
                        start=(P == 0), stop=(P == 1), perf_mode=DR,
                    )
                if gj == 0:
                    h = h1p.tile([128, 2, BBLK], F8, tag="h1a")
                    st_["h1"].setdefault(blk, []).append(h)
                elif gj == 2:
                    h = h1p.tile([128, 2, BBLK], F8, tag="h1b")
                    st_["h1"][blk].append(h)
                h = st_["h1"][blk][gj // 2]
                relu_drain(H1_ENG[(blk, gj // 2)], h[:, gj % 2, :], ps,
                           gb_b[gj])

            def touch_h1(blk):
                # PE observes the producing engines' ticks of both h1 pair
                # tiles, so L2 matmuls keep at most one sync wait.
                for h in st_["h1"][blk]:
                    _touch(nc, scratch, h[:, 1, :])

            def l2(blk):
                pp = p6p if blk == 1 else pmp
                ps = pp.tile([128, BBLK], F32,
                             tag="p6" if blk == 1 else "pmid")
                for P in range(2):
                    nc.tensor.matmul(
                        ps[:], w1_sl(P), st_["h1"][blk][P][:],
                        start=(P == 0), stop=(P == 1), perf_mode=DR,
                    )
                h2 = ap_.tile([HID, BBLK], BF, tag="h2")
                relu_drain("dve", h2[:], ps, b1_b)
                st_["h2"][blk] = h2

            def l3(blk):
                pp = p6p if blk == 1 else pmp
                ps = pp.tile([LAT, BBLK], F32,
                             tag="p6" if blk == 1 else "pmid")
                nc.tensor.matmul(ps[:], w2_sl, st_["h2"][blk][:], start=True,
                                 stop=True)
                z = ap_.tile([LAT, BBLK], BF, tag="z")
                relu_drain(Z_ENG[blk], z[:], ps, b2_b)
                st_["z"][blk] = z

            def l4(blk):
                pp = p6p if blk == 1 else pmp
                ps = pp.tile([HID, BBLK], F32,
                             tag="p6" if blk == 1 else "pmid")
                nc.tensor.matmul(ps[:], dw1_sl, st_["z"][blk][:], start=True,
                                 stop=True)
                d1 = ap_.tile([HID, BBLK], BF, tag="d1")
                relu_drain("act", d1[:], ps, db1_b)
                st_["d1"][blk] = d1
                # PE observes the ACT tick of d1 so L5 matmuls keep at most
                # one sync wait (their psum-slot wait).
                _touch(nc, scratch, d1)

            def l5(blk, j):
                pp = p6p if blk == 1 else pmp
                ps = pp.tile([128, BBLK], F32,
                             tag="p6" if blk == 1 else "pmid")
                nc.tensor.matmul(ps[:], dw2_sl(j), st_["d1"][blk][:],
                                 start=True, stop=True)
                if j == 0:
                    d2 = d2p.tile([128, 2, BBLK], F8, tag="d2")
                    st_["d2"][blk] = d2
                d2 = st_["d2"][blk]
                relu_drain("dve", d2[:, j, :], ps, db2_b[j])

            def l6(blk, bi, pool=None):
                # out[bi] = sigmoid(d2T[:, bi].T @ dw3 + db3), natural
                # layout; both the data matmul and the db3 broadcast
                # (ones-outer-product, hi+residual fp8 compensation) are
                # fp8 DoubleRow in one accumulation group.  Late l6 groups
                # borrow the (by then idle) pmid banks so four sigmoid
                # psums are in flight at once.
                if bi == 0:
                    ob = op_.tile([128, 4, S], BF, tag="ob")
                    st_["ob"][blk] = ob
                ps = (pool or (p6p if blk == 1 else pmp)).tile(
                    [128, S], F32, tag="p6" if blk == 1 else "pmid")
                nc.tensor.matmul(
                    ps[:], st_["d2"][blk][:, :, ts(bi, 128)], dw3_sl,
                    start=True, stop=False, perf_mode=DR,
                )
                nc.tensor.matmul(ps[:], ones_sl, db3_sl, start=False,
                                 stop=True, perf_mode=DR)
                nc.scalar.activation(st_["ob"][blk][:, bi, :], ps[:],
                                     AF.Sigmoid)

            def store(blk, h):
                # Half-block HWDGE stores, emitted right after the gating
                # sigmoid pair so the transfers overlap remaining compute.
                ob = st_["ob"][blk]
                nc.sync.dma_start(out=out_v[blk, :, 2 * h : 2 * h + 2],
                                  in_=ob[:, 2 * h : 2 * h + 2])

            # Software-pipelined emission: per-engine streams execute in
            # program order, so blk1's L1 is interleaved into blk0's
            # mid-layer chain, and the two blocks' L6/sigmoid groups are
            # interleaved so the final store's gating sigmoid runs early.
            # The first ACT op has no data deps: it absorbs the one-time
            # activation-table load (~1.3us) before real work arrives.
            # (DVE memset first so the source is initialized for CoreSim.)
            nc.vector.memset(wdump[0:1, 0:2], 0.0)
            _stouch(nc, sdump, wdump[0:1, 0:1])
            _touch(nc, scratch, xts[0][:, 0, 0, :])
            a8v = a_sb[:, 0:2560].bitcast(F8)
            _touch(nc, scratch, a8v)
            _vtouch(nc, vdump, a8v[:, 0:1])
            _stouch(nc, sdump, a8v[:, 0:1])
            for gj in range(4):
                l1(0, gj)
            _touch(nc, scratch, xts[1][:, 0, 0, :])
            for gj in range(4):
                l1(1, gj)
            touch_h1(0)
            l2(0)
            l3(0)
            l4(0)
            l5(0, 0)
            l5(0, 1)
            touch_h1(1)
            l2(1)
            l3(1)
            l4(1)
            l5(1, 0)
            l5(1, 1)
            _touch(nc, scratch, w8_sb[:, 0, :])
            _touch(nc, scratch, st_["d2"][0][:, 1, :])
            l6(0, 0)
            l6(0, 1)
            store(0, 0)
            _touch(nc, scratch, st_["d2"][1][:, 1, :])
            l6(1, 0)
            l6(1, 1)
            store(1, 0)
            l6(0, 2)
            l6(0, 3)
            store(0, 1)
            l6(1, 2)
            l6(1, 3)
            store(1, 1)

    return nc


def _get_program():
    if "prog" not in _CACHE:
        _CACHE["prog"] = _build_program()
    return _CACHE["prog"]


def _to_bf16(a):
    """Round-to-nearest-even fp32 -> bf16, returned as a uint16 view."""
    u = np.ascontiguousarray(a, dtype=np.float32).view(np.uint32)
    return (((u >> 16) + ((u >> 15) & 1)) & 0xFFFF).astype(np.uint16)


def _bf16_arr(u16):
    import ml_dtypes

    return u16.view(ml_dtypes.bfloat16)


def _f8(a):
    import ml_dtypes

    return np.asarray(a, dtype=np.float32).astype(ml_dtypes.float8_e4m3)


def _put_f32(pack_u16, rows, col2, vals):
    """Embed fp32 values as adjacent uint16 pairs at bf16 column col2."""
    v = np.asarray(vals, dtype=np.float32).view(np.uint32)
    pack_u16[:rows, col2] = (v & 0xFFFF).astype(np.uint16)
    pack_u16[:rows, col2 + 1] = (v >> 16).astype(np.uint16)


def _pack_params(inputs):
    import ml_dtypes

    gw = np.asarray(inputs["gw"], dtype=np.float32)
    gb = np.asarray(inputs["gb"], dtype=np.float32)
    idx = np.asarray(inputs["idx"], dtype=np.int64)
    mask = np.asarray(inputs["mask"], dtype=np.float32)
    w1 = np.asarray(inputs["w1"], dtype=np.float32)
    b1 = np.asarray(inputs["b1"], dtype=np.float32)
    w2 = np.asarray(inputs["w2"], dtype=np.float32)
    b2 = np.asarray(inputs["b2"], dtype=np.float32)
    dw1 = np.asarray(inputs["dw1"], dtype=np.float32)
    db1 = np.asarray(inputs["db1"], dtype=np.float32)
    dw2 = np.asarray(inputs["dw2"], dtype=np.float32)
    db2 = np.asarray(inputs["db2"], dtype=np.float32)
    dw3 = np.asarray(inputs["dw3"], dtype=np.float32)
    db3 = np.asarray(inputs["db3"], dtype=np.float32)

    g, k = idx.shape
    assert g == G

    # Fold gather + grouped Dense(1) into a dense [S, GP] matrix.
    a_mat = np.zeros((S, GP), dtype=np.float32)
    gwm = (gw * mask).astype(np.float32)
    cols = np.repeat(np.arange(g, dtype=np.int64), k)
    np.add.at(a_mat, (idx.reshape(-1), cols), gwm.reshape(-1))

    # apack raw bytes [p, APR_COLS]: A+w1 fp8 in DoubleRow layout ([P, i,
    # gj|4=w1, m] with rows s/g = P*256 + i*128 + p), then fp32 biases and
    # bf16 mid weights as raw bytes.
    apack = np.zeros((128, APR_COLS), dtype=np.uint8)
    a8 = _f8(a_mat).reshape(2, 2, 128, 4, 128)        # [P, i, p, gj, m]
    w1_pad = np.zeros((GP, HID), dtype=np.float32)
    w1_pad[:g] = w1
    w18 = _f8(w1_pad).reshape(2, 2, 128, 128)         # [P, i, p, m]
    a_full = np.zeros((128, 2, 2, 5, 128), dtype=ml_dtypes.float8_e4m3)
    a_full[:, :, :, 0:4, :] = a8.transpose(2, 0, 1, 3, 4)
    a_full[:, :, :, 4, :] = w18.transpose(2, 0, 1, 3)
    apack[:, A_BASE : A_BASE + 2560] = \
        a_full.reshape(128, 2560).view(np.uint8)

    def put_f32_bytes(col, vals, rows=128):
        v = np.asarray(vals, dtype=np.float32)
        apack[:rows, col : col + 4] = v.view(np.uint8).reshape(rows, 4)

    gb_pad = np.zeros(GP, np.float32)
    gb_pad[:g] = gb
    for i in range(4):
        put_f32_bytes(BIAS_BASE + 4 * i, gb_pad[i * 128 : (i + 1) * 128])
    put_f32_bytes(BIAS_BASE + 16, b1)
    put_f32_bytes(BIAS_BASE + 20, b2, rows=LAT)
    put_f32_bytes(BIAS_BASE + 24, db1)
    put_f32_bytes(BIAS_BASE + 28, db2[:128])
    put_f32_bytes(BIAS_BASE + 32, db2[128:])
    apack[:, W2_BASE : W2_BASE + 2 * LAT] = \
        _to_bf16(w2).view(np.uint8).reshape(128, 2 * LAT)
    apack[:LAT, DW1_BASE : DW1_BASE + 2 * HID] = \
        _to_bf16(dw1).view(np.uint8).reshape(LAT, 2 * HID)
    apack[:, DW2_BASE : DW2_BASE + 512] = \
        _to_bf16(dw2).view(np.uint8).reshape(128, 512)

    # w8pack [p, i, free]: dw3 rows k = i*128 + p; ones + compensated db3
    # rows on partition 0.
    w8 = np.zeros((128, 2, W8_FREE), dtype=ml_dtypes.float8_e4m3)
    dw38 = _f8(dw3).reshape(2, 128, S)                # [i, p, s]
    w8[:, :, DW3_OFF : DW3_OFF + S] = dw38.transpose(1, 0, 2)
    w8[0, :, ONES_OFF : ONES_OFF + 128] = np.float32(1.0)
    db3_hi = _f8(db3)
    db3_lo = _f8(db3 - db3_hi.astype(np.float32))
    w8[0, 0, DB3_OFF : DB3_OFF + S] = db3_hi
    w8[0, 1, DB3_OFF : DB3_OFF + S] = db3_lo

    return apack, w8


def kernel(**inputs) -> np.ndarray:
    global last_results

    x = np.asarray(inputs["x"], dtype=np.float32)
    assert x.shape == (B, S), x.shape
    x8 = _f8(x)
    apack, w8pack = _pack_params(inputs)

    in_maps = [
        {"xt": np.ascontiguousarray(x8[c * BC : (c + 1) * BC].T),
         "apack": apack, "w8pack": w8pack}
        for c in range(NCORES)
    ]

    nc = _get_program()
    trace = os.environ.get("KERNEL_TRACE", "0") == "1"
    res = run_bass_kernel_spmd(nc, in_maps, list(range(NCORES)), trace=trace)
    last_results = res
    out = np.concatenate(
        [np.asarray(r["out"]) for r in res.results], axis=0
    )
    return out.astype(np.float32)


if __name__ == "__main__":
    rng = np.random.RandomState(0)
    demo = {
        "x": rng.rand(B, S).astype(np.float32),
        "gw": rng.randn(G, 30).astype(np.float32) * 0.18,
        "gb": rng.randn(G).astype(np.float32) * 0.1,
        "idx": rng.randint(0, S, (G, 30)).astype(np.int32),
        "mask": (rng.rand(G, 30) > 0.5).astype(np.float32),
        "w1": rng.randn(G, HID).astype(np.float32) * 0.04,
        "b1": rng.randn(HID).astype(np.float32) * 0.1,
        "w2": rng.randn(HID, LAT).astype(np.float32) * 0.09,
        "b2": rng.randn(LAT).astype(np.float32) * 0.1,
        "dw1": rng.randn(LAT, HID).astype(np.float32) * 0.18,
        "db1": rng.randn(HID).astype(np.float32) * 0.1,
        "dw2": rng.randn(HID, HID2).astype(np.float32) * 0.09,
        "db2": rng.randn(HID2).astype(np.float32) * 0.1,
        "dw3": rng.randn(HID2, S).astype(np.float32) * 0.06,
        "db3": rng.randn(S).astype(np.float32) * 0.1,
    }
    out = kernel(**demo)

    # numpy check of the dense-folded network
    x = demo["x"].astype(np.float64)
    a_mat = np.zeros((S, GP))
    gwm = demo["gw"] * demo["mask"]
    cols = np.repeat(np.arange(G), 30)
    np.add.at(a_mat, (demo["idx"].reshape(-1).astype(np.int64), cols),
              gwm.reshape(-1).astype(np.float64))
    gb_pad = np.zeros(GP)
    gb_pad[:G] = demo["gb"]
    w1_pad = np.zeros((GP, HID))
    w1_pad[:G] = demo["w1"]
    relu = lambda v: np.maximum(v, 0)
    h = relu(x @ a_mat + gb_pad)
    h = relu(h @ w1_pad + demo["b1"])
    z = relu(h @ demo["w2"] + demo["b2"])
    d = relu(z @ demo["dw1"] + demo["db1"])
    d = relu(d @ demo["dw2"] + demo["db2"])
    exp = 1 / (1 + np.exp(-(d @ demo["dw3"] + demo["db3"])))
    err = np.linalg.norm(out - exp) / np.linalg.norm(exp)
    print("out", out.shape, out.dtype, float(out.mean()))
    print("demo rel_fro vs numpy:", err)
